# revision 1
# baseline (speedup 1.0000x reference)
"""DirGCNConv on 8 Trainium2 NeuronCores via Bass/Tile.

out = (1-a)*(Dout^-1/2 A Din^-1/2 x) @ Wsrc.T + a*(Din^-1/2 A.T Dout^-1/2 x) @ Wdst.T + bias

Key identity: the per-edge weight w[e] = ao[row[e]] * bi[col[e]] is separable
(ao = invsqrt(deg_out), bi = invsqrt(deg_in)), so

  agg_src[i] = ao[i] * sum_{e: row=i} (bi*x)[col[e]]
  agg_dst[j] = bi[j] * sum_{e: col=j} (ao*x)[row[e]]

Each direction is a pure gather + scatter-add of 512B rows, which TRN2 has
native SWDGE instructions for (dma_gather / dma_scatter_add, int16 indices).

Sharding: nodes are split into 8 contiguous ranges (12500/core); each core
owns the edges whose *destination* falls in its range, so scatter-adds are
core-local (int16-safe). Gathers read from a replicated copy of x (and of the
invsqrt "pad" arrays built on device), windowed into 32768-row slices to stay
within int16 index range. All floating-point math happens on device; the host
only reorganizes indices (bucketing, rowptrs, int16 conversion).
"""

import os

import numpy as np
from contextlib import ExitStack

# recover cleanly if a previous run left the NeuronCores wedged (must be set
# before the Neuron runtime initializes in this process)
os.environ.setdefault("NEURON_RT_RESET_CORES", "1")

N = 100000
E = 600000
D = 128
NCORES = 8
ALPHA = 0.5

WIN = 32768          # gather source window (int16 index range)
CALL = 1024          # max tokens per dma_gather/dma_scatter_add call (ucode
                     # wedges above this: 64 idx columns x 16 lanes)
TILE_TOK = 4096      # tokens per SBUF tile (several gather calls fill one)
SEG = 4096           # prescale segment rows (SEG = 128 * SEGG)
SEGG = SEG // 128


def _cfg_for(n_nodes):
    nw = (n_nodes + WIN - 1) // WIN
    nloc = n_nodes // NCORES
    return dict(
        N=n_nodes,
        NW=nw,
        NLOC=nloc,
        NOUT_BLK=(nloc + 127) // 128,
    )


def _wrap_idx(arr):
    """int16 array [B] -> [128, B//16] wrapped-16 layout, replicated 8x
    across partition blocks (one replica per gpsimd ucode core)."""
    b = arr.shape[0]
    assert b % 16 == 0
    t = arr.reshape(b // 16, 16).T.copy()          # [16, B/16]
    return np.tile(t, (8, 1)).astype(np.int16)     # [128, B/16]


def _chunks(total):
    out = []
    off = 0
    while off < total:
        c = min(CH, total - off)
        out.append((off, c))
        off += c
    return out


def _prep_host(x, edge_index, W_src, b_src, W_dst, b_dst, cfg):
    """Pure index reorganization on host -> per-core input maps."""
    n, nw, nloc = cfg["N"], cfg["NW"], cfg["NLOC"]
    row = np.asarray(edge_index[0], dtype=np.int64)
    col = np.asarray(edge_index[1], dtype=np.int64)

    # rowptrs (cumulative histograms; device computes deg = hi - lo)
    rp_row = np.zeros(n + 1, dtype=np.int64)
    rp_row[1:] = np.cumsum(np.bincount(row, minlength=n))
    rp_col = np.zeros(n + 1, dtype=np.int64)
    rp_col[1:] = np.cumsum(np.bincount(col, minlength=n))

    def rp_prescale(rp):
        # rowptr pairs arranged in the device prescale traversal order:
        # per window: full SEG segments (node at (p,g) = base+si*SEG+p*SEGG+g),
        # then t1 wrap-(g p) columns, then one partial column of t2 rows
        cols = []
        for w in range(nw):
            base = w * WIN
            rows_w = min(WIN, n - base)
            nseg = rows_w // SEG
            for si in range(nseg):
                cols.append(base + si * SEG
                            + np.arange(128)[:, None] * SEGG
                            + np.arange(SEGG)[None, :])
            r0 = base + nseg * SEG
            rem = rows_w - nseg * SEG
            t1 = rem // 128
            if t1:
                cols.append(r0 + np.arange(t1)[None, :] * 128
                            + np.arange(128)[:, None])
            t2 = rem - t1 * 128
            if t2:
                c = r0 + t1 * 128 + np.arange(128)[:, None]
                cols.append(np.where(c < base + rows_w, c, n))
        idx = np.concatenate(cols, axis=1)
        idx = np.minimum(idx, n)
        lo = rp[idx].astype(np.int32)
        hi = rp[np.minimum(idx + 1, n)].astype(np.int32)
        return lo, hi

    rpa_lo, rpa_hi = rp_prescale(rp_row)   # deg_out -> a (xa scale)
    rpb_lo, rpb_hi = rp_prescale(rp_col)   # deg_in  -> b (xb scale)

    def rp_local(rp, c):
        # [128, NBLK]: node at (p, k) is c*nloc + k*128 + p (clamped -> deg 0)
        nblk = cfg["NBLK"]
        idx = (np.arange(nblk)[None, :] * 128 + np.arange(128)[:, None])
        valid = idx < nloc
        idx = np.minimum(c * nloc + idx, n)
        lo = rp[idx]
        hi = rp[np.minimum(idx + 1, n)]
        hi = np.where(valid, hi, lo)
        return lo.astype(np.int32), hi.astype(np.int32)

    # Bucket edges by (dest core, source window); dir1 dest=row src=col,
    # dir2 dest=col src=row. Within each bucket, edges are ordered by
    # occurrence-rank per dest ("sub-batches"): the k-th edge of every dest
    # goes to sub-batch k, so a scatter call confined to one sub-batch never
    # has duplicate dest rows (hardware dma_scatter_add loses concurrent
    # same-row adds). Gathers chunk freely at CH; scatter calls additionally
    # cut at sub-batch boundaries. Pads scatter to a rotating trash area.
    def bucket(dest, src):
        core = dest // nloc
        win = src >> 15
        per = {}
        kc = {}
        for c in range(NCORES):
            mc = core == c
            for w in range(nw):
                m = mc & (win == w)
                d = (dest[m] - c * nloc).astype(np.int64)
                s_ = (src[m] - w * WIN).astype(np.int64)
                o = np.argsort(d, kind="stable")
                d, s_ = d[o], s_[o]
                if len(d):
                    new = np.r_[True, d[1:] != d[:-1]]
                    starts = np.nonzero(new)[0]
                    gi = np.cumsum(new) - 1
                    occ = np.arange(len(d)) - starts[gi]
                    o2 = np.argsort(occ, kind="stable")
                    d, s_, occ = d[o2], s_[o2], occ[o2]
                    kcounts = np.bincount(occ)
                else:
                    occ = d
                    kcounts = np.zeros(0, np.int64)
                per[(c, w)] = (d, s_, occ)
                kc[(c, w)] = kcounts

        # static sub-batch sizes per window (shared across cores)
        sbk = []
        for w in range(nw):
            nb = max(len(kc[(c, w)]) for c in range(NCORES))
            sizes = []
            for k in range(nb):
                mx = max((kc[(c, w)][k] if k < len(kc[(c, w)]) else 0)
                         for c in range(NCORES))
                sizes.append(((mx + 127) // 128) * 128)
            sbk.append([s for s in sizes if s > 0])

        # chunk plan per window: SBUF tiles, gather calls (<=CALL tokens),
        # scatter pieces (<=CALL, cut additionally at sub-batch boundaries)
        windows = []
        start = 0
        for w in range(nw):
            length = sum(sbk[w])
            tiles = [(t, min(TILE_TOK, length - t))
                     for t in range(0, length, TILE_TOK)]
            gathers = [(a, min(CALL, length - a))
                       for a in range(0, length, CALL)]
            cuts = set(range(0, length, CALL))
            cuts.add(length)
            b = 0
            for s_sz in sbk[w]:
                cuts.add(b)
                b += s_sz
            cuts = sorted(cuts)
            bat_starts = []
            b = 0
            for s_sz in sbk[w]:
                bat_starts.append(b)
                b += s_sz
            def _parity(a):
                p = 0
                for bi, bs in enumerate(bat_starts):
                    if a >= bs:
                        p = bi & 1
                return p
            pieces = [(cuts[i], cuts[i + 1] - cuts[i], _parity(cuts[i]))
                      for i in range(len(cuts) - 1)]
            windows.append(dict(start=start, length=length, tiles=tiles,
                                gathers=gathers, pieces=pieces, sbk=sbk[w]))
            start += length

        # max pads inside any scatter piece -> trash size
        max_pad = 0
        for w, wd in enumerate(windows):
            for c in range(NCORES):
                b = 0
                for k, s_sz in enumerate(sbk[w]):
                    cnt = kc[(c, w)][k] if k < len(kc[(c, w)]) else 0
                    for (a, ln, _pp) in wd["pieces"]:
                        if a >= b and a < b + s_sz:
                            pad = max(0, (a + ln) - max(a, b + int(cnt)))
                            max_pad = max(max_pad, pad)
                    b += s_sz
        tr = max(128, ((max_pad + 127) // 128) * 128)

        # build per-core padded token streams
        g_out, s_out = [], []
        for c in range(NCORES):
            gs_all, ss_all = [], []
            for w in range(nw):
                d, s_, occ = per[(c, w)]
                kcounts = kc[(c, w)]
                pos = 0
                for k, s_sz in enumerate(sbk[w]):
                    cnt = int(kcounts[k]) if k < len(kcounts) else 0
                    gs = s_[pos:pos + cnt]
                    ss = d[pos:pos + cnt]
                    pad = s_sz - cnt
                    if pad:
                        gs = np.concatenate([gs, np.zeros(pad, np.int64)])
                        ss = np.concatenate(
                            [ss, nloc + (np.arange(pad) % tr)])
                    gs_all.append(gs)
                    ss_all.append(ss)
                    pos += cnt
            g_out.append(np.concatenate(gs_all).astype(np.int16))
            s_out.append(np.concatenate(ss_all).astype(np.int16))
        return dict(windows=windows, total=start, tr=tr), g_out, s_out

    plan1, g1, s1 = bucket(row, col)
    plan2, g2, s2 = bucket(col, row)

    tr = max(plan1["tr"], plan2["tr"])
    cfg["TR"] = tr
    cfg["NLOC_PAD"] = ((nloc + tr + 127) // 128) * 128
    cfg["NBLK"] = cfg["NLOC_PAD"] // 128

    ident = np.eye(128, dtype=np.float32)
    wsrcT = np.ascontiguousarray(np.asarray(W_src, np.float32).T)
    wdstT = np.ascontiguousarray(np.asarray(W_dst, np.float32).T)
    xf = np.ascontiguousarray(np.asarray(x, np.float32))

    in_maps = []
    for c in range(NCORES):
        rp1_lo, rp1_hi = rp_local(rp_row, c)   # dir1 dest deg (deg_out) -> a_vec
        rp2_lo, rp2_hi = rp_local(rp_col, c)   # dir2 dest deg (deg_in)  -> b_vec
        in_maps.append({
            "x": xf,
            "wsrcT": wsrcT,
            "wdstT": wdstT,
            "ident": ident,
            "bsrc": np.asarray(b_src, np.float32),
            "bdst": np.asarray(b_dst, np.float32),
            "g1": _wrap_idx(g1[c]), "s1": _wrap_idx(s1[c]),
            "g2": _wrap_idx(g2[c]), "s2": _wrap_idx(s2[c]),
            "rpa_lo": rpa_lo, "rpa_hi": rpa_hi,
            "rpb_lo": rpb_lo, "rpb_hi": rpb_hi,
            "rp1_lo": rp1_lo, "rp1_hi": rp1_hi,
            "rp2_lo": rp2_lo, "rp2_hi": rp2_hi,
        })
    return in_maps, plan1, plan2


def _build(cfg, plan1, plan2, debug=False):
    import concourse.bass as bass
    import concourse.tile as tile
    from concourse import bacc, mybir

    dt = mybir.dt
    n, nw = cfg["N"], cfg["NW"]
    nloc, nloc_pad, nblk = cfg["NLOC"], cfg["NLOC_PAD"], cfg["NBLK"]
    s1_cols = plan1["total"] // 16
    s2_cols = plan2["total"] // 16

    nc = bacc.Bacc("TRN2", target_bir_lowering=False, debug=False,
                   num_devices=NCORES)

    x = nc.dram_tensor("x", [n, D], dt.float32, kind="ExternalInput")
    wsrcT = nc.dram_tensor("wsrcT", [D, D], dt.float32, kind="ExternalInput")
    wdstT = nc.dram_tensor("wdstT", [D, D], dt.float32, kind="ExternalInput")
    ident = nc.dram_tensor("ident", [D, D], dt.float32, kind="ExternalInput")
    bsrc = nc.dram_tensor("bsrc", [D], dt.float32, kind="ExternalInput")
    bdst = nc.dram_tensor("bdst", [D], dt.float32, kind="ExternalInput")
    g1 = nc.dram_tensor("g1", [128, s1_cols], dt.int16, kind="ExternalInput")
    s1 = nc.dram_tensor("s1", [128, s1_cols], dt.int16, kind="ExternalInput")
    g2 = nc.dram_tensor("g2", [128, s2_cols], dt.int16, kind="ExternalInput")
    s2 = nc.dram_tensor("s2", [128, s2_cols], dt.int16, kind="ExternalInput")
    # prescale traversal: per window, list of (kind, rows0, nrows, ncols)
    presched = []
    pcols = 0
    for w in range(nw):
        base = w * WIN
        rows_w = min(WIN, n - base)
        nseg = rows_w // SEG
        steps = []
        for si in range(nseg):
            steps.append(("seg", base + si * SEG, SEG, SEGG))
            pcols += SEGG
        r0 = base + nseg * SEG
        rem = rows_w - nseg * SEG
        t1 = rem // 128
        if t1:
            steps.append(("t1", r0, t1 * 128, t1))
            pcols += t1
        t2 = rem - t1 * 128
        if t2:
            steps.append(("t2", r0 + t1 * 128, t2, 1))
            pcols += 1
        presched.append(steps)

    rpa_lo = nc.dram_tensor("rpa_lo", [128, pcols], dt.int32, kind="ExternalInput")
    rpa_hi = nc.dram_tensor("rpa_hi", [128, pcols], dt.int32, kind="ExternalInput")
    rpb_lo = nc.dram_tensor("rpb_lo", [128, pcols], dt.int32, kind="ExternalInput")
    rpb_hi = nc.dram_tensor("rpb_hi", [128, pcols], dt.int32, kind="ExternalInput")
    rp1_lo = nc.dram_tensor("rp1_lo", [128, nblk], dt.int32, kind="ExternalInput")
    rp1_hi = nc.dram_tensor("rp1_hi", [128, nblk], dt.int32, kind="ExternalInput")
    rp2_lo = nc.dram_tensor("rp2_lo", [128, nblk], dt.int32, kind="ExternalInput")
    rp2_hi = nc.dram_tensor("rp2_hi", [128, nblk], dt.int32, kind="ExternalInput")
    out = nc.dram_tensor("out", [nloc, D], dt.float32, kind="ExternalOutput")

    kind = dict(kind="ExternalOutput") if debug else {}
    xbw, xaw = [], []
    for w in range(nw):
        rows_w = min(WIN, n - w * WIN)
        xbw.append(nc.dram_tensor(f"xb{w}", [rows_w, D], dt.float32, **kind))
        xaw.append(nc.dram_tensor(f"xa{w}", [rows_w, D], dt.float32, **kind))
    agg1 = [nc.dram_tensor(f"agg1{p}", [nloc_pad, D], dt.float32, **kind)
            for p in range(2)]
    agg2 = [nc.dram_tensor(f"agg2{p}", [nloc_pad, D], dt.float32, **kind)
            for p in range(2)]
    out1d = nc.dram_tensor("out1d", [nloc_pad, D], dt.float32)

    with tile.TileContext(nc) as tc, ExitStack() as ctx:
        const = ctx.enter_context(tc.tile_pool(name="const", bufs=1))

        # --- constants ---
        wsrcT_sb = const.tile([D, D], dt.float32, tag="wsrc")
        nc.sync.dma_start(wsrcT_sb[:], wsrcT.ap())
        wdstT_sb = const.tile([D, D], dt.float32, tag="wdst")
        nc.sync.dma_start(wdstT_sb[:], wdstT.ap())
        ident_sb = const.tile([D, D], dt.float32, tag="ident")
        nc.sync.dma_start(ident_sb[:], ident.ap())

        brow = const.tile([1, 2 * D], dt.float32, tag="brow")
        nc.sync.dma_start(brow[:, 0:D], bsrc.ap().unsqueeze(0))
        nc.sync.dma_start(brow[:, D:2 * D], bdst.ap().unsqueeze(0))
        bsum = const.tile([1, D], dt.float32, tag="bsum")
        nc.vector.tensor_add(bsum[:], brow[:, 0:D], brow[:, D:2 * D])
        nc.vector.tensor_scalar_mul(bsum[:], bsum[:], ALPHA)
        bias_bc = const.tile([D, D], dt.float32, tag="biasbc")
        nc.gpsimd.partition_broadcast(bias_bc[:], bsum[:])

        # resident index arrays
        g1_sb = const.tile([128, s1_cols], dt.int16, tag="g1")
        nc.sync.dma_start(g1_sb[:], g1.ap())
        s1_sb = const.tile([128, s1_cols], dt.int16, tag="s1")
        nc.sync.dma_start(s1_sb[:], s1.ap())
        g2_sb = const.tile([128, s2_cols], dt.int16, tag="g2")
        nc.sync.dma_start(g2_sb[:], g2.ap())
        s2_sb = const.tile([128, s2_cols], dt.int16, tag="s2")
        nc.sync.dma_start(s2_sb[:], s2.ap())

        def invsqrt_chain(pool, lo_ap, hi_ap, cols, tag, scale=None,
                          res_pool=None):
            """deg = hi-lo; return invsqrt(max(deg,1)) * (deg>0) [* scale]."""
            res_pool = res_pool or pool
            lo_t = pool.tile([128, cols], dt.int32, tag=tag + "lo")
            nc.sync.dma_start(lo_t[:], lo_ap)
            hi_t = pool.tile([128, cols], dt.int32, tag=tag + "hi")
            nc.sync.dma_start(hi_t[:], hi_ap)
            deg_i = pool.tile([128, cols], dt.int32, tag=tag + "di")
            nc.vector.tensor_sub(deg_i[:], hi_t[:], lo_t[:])
            deg_f = pool.tile([128, cols], dt.float32, tag=tag + "df")
            nc.vector.tensor_copy(deg_f[:], deg_i[:])
            mask = pool.tile([128, cols], dt.float32, tag=tag + "mk")
            mul = scale if scale is not None else 1.0
            nc.vector.tensor_scalar(mask[:], deg_f[:], 1.0, mul,
                                    mybir.AluOpType.min, mybir.AluOpType.mult)
            dmax = pool.tile([128, cols], dt.float32, tag=tag + "dm")
            nc.vector.tensor_scalar_max(dmax[:], deg_f[:], 1.0)
            rec = pool.tile([128, cols], dt.float32, tag=tag + "rc")
            nc.vector.reciprocal(rec[:], dmax[:])
            sq = pool.tile([128, cols], dt.float32, tag=tag + "sq")
            nc.scalar.sqrt(sq[:], rec[:])
            res = res_pool.tile([128, cols], dt.float32, tag=tag + "rs")
            nc.vector.tensor_mul(res[:], sq[:], mask[:])
            return res

        # scale vectors (dest-side 0.5-folded; gather-side full arrays);
        # chain intermediates live in a scratch pool freed before the big
        # streaming pools open
        with tc.tile_pool(name="chainscratch", bufs=1) as csp:
            a_vec = invsqrt_chain(csp, rp1_lo.ap(), rp1_hi.ap(), nblk, "av",
                                  scale=1.0 - ALPHA, res_pool=const)
            b_vec = invsqrt_chain(csp, rp2_lo.ap(), rp2_hi.ap(), nblk, "bv",
                                  scale=ALPHA, res_pool=const)
            b_full = invsqrt_chain(csp, rpb_lo.ap(), rpb_hi.ap(), pcols,
                                   "bf", res_pool=const)
            a_full = invsqrt_chain(csp, rpa_lo.ap(), rpa_hi.ap(), pcols,
                                   "af", res_pool=const)

        edge_pool = ctx.enter_context(tc.tile_pool(name="edge", bufs=3))

        # --- prescale: xb = b (.) x, xa = a (.) x, streamed per window ---
        with tc.tile_pool(name="prescale", bufs=2) as pp:
            ccur = 0
            for w in range(nw):
                base = w * WIN
                for kind_, r0, nrows, ncols in presched[w]:
                    cs = slice(ccur, ccur + ncols)
                    ccur += ncols
                    if kind_ == "t2":
                        xt = pp.tile([nrows, D], dt.float32, tag="pxt2")
                        nc.sync.dma_start(xt[:], x.ap()[r0:r0 + nrows, :])
                        for dest, sv in ((xbw[w], b_full), (xaw[w], a_full)):
                            ot = pp.tile([nrows, D], dt.float32, tag="pot2")
                            nc.scalar.mul(ot[:], xt[:], sv[0:nrows, cs])
                            nc.sync.dma_start(
                                dest.ap()[r0 - base:r0 - base + nrows, :],
                                ot[:])
                        continue
                    wrap = "(p g) d -> p g d" if kind_ == "seg" \
                        else "(g p) d -> p g d"
                    xs = x.ap()[r0:r0 + nrows, :].rearrange(wrap, p=128)
                    xt = pp.tile([128, ncols, D], dt.float32, tag="pxt")
                    nc.sync.dma_start(xt[:], xs)
                    for dest, sv in ((xbw[w], b_full), (xaw[w], a_full)):
                        ex = pp.tile([128, ncols, D], dt.float32, tag="pex")
                        nc.vector.tensor_copy(ex[:, :, 0:1],
                                              sv[:, cs].unsqueeze(2))
                        ww = 1
                        while ww < D:
                            nc.vector.tensor_copy(ex[:, :, ww:2 * ww],
                                                  ex[:, :, 0:ww])
                            ww *= 2
                        nc.vector.tensor_mul(ex[:], ex[:], xt[:])
                        dv = dest.ap()[r0 - base:r0 - base + nrows, :] \
                            .rearrange(wrap, p=128)
                        nc.sync.dma_start(dv, ex[:])

        # --- zero the accumulators ---
        with tc.tile_pool(name="zero", bufs=1) as zp:
            zt = zp.tile([128, 2048], dt.float32)
            nc.vector.memset(zt[:], 0.0)
            for agg in (*agg1, *agg2):
                flat = agg.ap().rearrange("(p r) d -> p (r d)", p=128)
                total = flat.shape[1]
                off = 0
                while off < total:
                    c = min(2048, total - off)
                    nc.sync.dma_start(flat[:, off:off + c], zt[:, 0:c])
                    off += c

        # --- edge phases: gather prescaled rows, scatter-add to agg ---
        def edge_phase(plan, g_sb, s_sb, srcw, agg, pools):
            (xpool,) = pools
            for w, wd in enumerate(plan["windows"]):
                xs = srcw[w].ap()
                gathers, pieces = wd["gathers"], wd["pieces"]
                gi_i = 0
                pi = 0
                for t0, tl in wd["tiles"]:
                    xt = xpool.tile([128, tl // 128, D], dt.float32, tag="xt")
                    while gi_i < len(gathers) and \
                            gathers[gi_i][0] + gathers[gi_i][1] <= t0 + tl:
                        a, ln = gathers[gi_i]
                        o = wd["start"] + a
                        gi = g_sb[:, o // 16:(o + ln) // 16]
                        g0 = (a - t0) // 128
                        ge = g0 + ln // 128
                        nc.gpsimd.dma_gather(xt[:, g0:ge, :], xs, gi, ln, ln,
                                             D)
                        gi_i += 1
                    # scatter pieces owned by this tile (conflict-free within
                    # each call: <=CALL tokens, unique dest rows)
                    while pi < len(pieces) and \
                            pieces[pi][0] + pieces[pi][1] <= t0 + tl:
                        a, ln, par = pieces[pi]
                        so = wd["start"] + a
                        si = s_sb[:, so // 16:(so + ln) // 16]
                        g0 = (a - t0) // 128
                        nc.gpsimd.dma_scatter_add(
                            agg[par].ap(), xt[:, g0:g0 + ln // 128, :], si,
                            ln, ln, D)
                        pi += 1

        # --- per-block matmul: out_blk = scale (.) agg_blk @ WT ---
        def mm_block(pools, agg, k, scale_vec, wT_sb):
            mpool, psum = pools
            ab = mpool.tile([128, D], dt.float32, tag="ab")
            nc.sync.dma_start(ab[:], agg[0].ap()[k * 128:(k + 1) * 128, :])
            ab1 = mpool.tile([128, D], dt.float32, tag="ab1")
            nc.sync.dma_start(ab1[:], agg[1].ap()[k * 128:(k + 1) * 128, :])
            abs_ = mpool.tile([128, D], dt.float32, tag="abs")
            nc.vector.tensor_add(abs_[:], ab[:], ab1[:])
            sc = mpool.tile([128, D], dt.float32, tag="sc")
            nc.scalar.mul(sc[:], abs_[:], scale_vec[:, k:k + 1])
            tp = psum.tile([128, D], dt.float32, tag="tp")
            nc.tensor.transpose(tp[:], sc[:], ident_sb[:])
            aT = mpool.tile([128, D], dt.float32, tag="aT")
            nc.vector.tensor_copy(aT[:], tp[:])
            om = psum.tile([128, D], dt.float32, tag="om")
            nc.tensor.matmul(om[:], lhsT=aT[:], rhs=wT_sb[:], start=True,
                             stop=True)
            return om

        edge_phase(plan1, g1_sb, s1_sb, xbw, agg1, (edge_pool,))

        with tc.tile_pool(name="mm1", bufs=3) as mp1, \
             tc.tile_pool(name="ps1", bufs=4, space="PSUM") as ps1:
            for k in range(cfg["NOUT_BLK"]):
                om = mm_block((mp1, ps1), agg1, k, a_vec, wsrcT_sb)
                o1 = mp1.tile([128, D], dt.float32, tag="o1")
                nc.vector.tensor_copy(o1[:], om[:])
                nc.sync.dma_start(out1d.ap()[k * 128:(k + 1) * 128, :], o1[:])

        edge_phase(plan2, g2_sb, s2_sb, xaw, agg2, (edge_pool,))

        with tc.tile_pool(name="mm2", bufs=3) as mp2, \
             tc.tile_pool(name="ps2", bufs=4, space="PSUM") as ps2:
            for k in range(cfg["NOUT_BLK"]):
                om = mm_block((mp2, ps2), agg2, k, b_vec, wdstT_sb)
                o1r = mp2.tile([128, D], dt.float32, tag="o1r")
                nc.sync.dma_start(o1r[:], out1d.ap()[k * 128:(k + 1) * 128, :])
                tmp = mp2.tile([128, D], dt.float32, tag="tmp")
                nc.vector.tensor_add(tmp[:], o1r[:], bias_bc[:])
                fin = mp2.tile([128, D], dt.float32, tag="fin")
                nc.vector.tensor_add(fin[:], tmp[:], om[:])
                rows = min(128, nloc - k * 128)
                nc.sync.dma_start(out.ap()[k * 128:k * 128 + rows, :],
                                  fin[0:rows, :])

    nc.compile()
    return nc


def _install_ntff_shim():
    """This image's antenv lacks axon_hooks; inject it so trace=True works."""
    import sys
    import types
    try:
        from antenv import axon_hooks  # noqa: F401
        return
    except ImportError:
        pass
    try:
        import antenv
        from trn_agent_boot.trn_boot import _ntff_profile_via_ctypes
        mod = types.ModuleType("antenv.axon_hooks")
        holder = [None]
        mod.set_axon_ntff_profile_hook = lambda h: holder.__setitem__(0, h)
        mod.get_axon_ntff_profile_hook = lambda: holder[0]
        sys.modules["antenv.axon_hooks"] = mod
        antenv.axon_hooks = mod
        mod.set_axon_ntff_profile_hook(
            _ntff_profile_via_ctypes("/opt/axon/libaxon_pjrt.so"))
    except Exception as e:  # profiling is best-effort
        print("ntff shim failed:", e)


def _run(nc, in_maps, trace=False):
    from concourse.bass_utils import run_bass_kernel_spmd
    kw = {}
    if trace:
        _install_ntff_shim()
        kw = dict(trace=True, trace_cores=list(range(NCORES)))
    return run_bass_kernel_spmd(nc, in_maps, list(range(NCORES)), **kw)


def kernel(x, edge_index, W_src, b_src, W_dst, b_dst, _trace=False,
           _return_result=False):
    cfg = _cfg_for(x.shape[0])
    in_maps, plan1, plan2 = _prep_host(x, edge_index, W_src, b_src, W_dst,
                                       b_dst, cfg)
    nc = _build(cfg, plan1, plan2)
    res = _run(nc, in_maps, trace=_trace)
    out = np.concatenate([res.results[c]["out"] for c in range(NCORES)],
                         axis=0)
    if _return_result:
        return out, res
    return out



# revision 4
# speedup vs baseline: 2.2767x; 2.2767x over previous
"""DirGCNConv on 8 Trainium2 NeuronCores via Bass/Tile (v2: scatter-free).

out = (1-a)*(Dout^-1/2 A Din^-1/2 x) @ Wsrc.T + a*(Din^-1/2 A.T Dout^-1/2 x) @ Wdst.T + bias

Per-edge weight separates: w[e] = ao[row[e]] * bi[col[e]], so each direction
is agg[dest] = Sum_{edges} prescaled_x[src], then a per-dest scale + matmul.

v2 strategy (vs v1 gather+scatter-add): edges are sorted by *destination*;
gathered source rows (bf16, dma_gather) are reduced per dest block with
one-hot segment matmuls on the Tensor engine accumulating in PSUM, then
folded into an SBUF-resident [feat x dest] accumulator. This removes all
dma_scatter_add calls — the GpSimd descriptor-generation engine (the
bottleneck) only runs gathers.

SPMD: one program for all 8 cores, so the chunk/matmul schedule is static:
each (window, dest-block) segment is padded to the max token count over
cores. Per-core data (gather indices, one-hot dest columns) differs only in
tensor contents.
"""

import os

import numpy as np
from contextlib import ExitStack

os.environ.setdefault("NEURON_RT_RESET_CORES", "1")

N = 100000
E = 600000
D = 128
NCORES = 8
ALPHA = 0.5

WIN = 25000          # gather source window rows (int16 range allows 32768)
CALL = 1024          # max tokens per dma_gather call
SEG = 2048           # prescale segment rows
SEGG = SEG // 128


def _cfg_for(n_nodes):
    nw = (n_nodes + WIN - 1) // WIN
    nloc = n_nodes // NCORES
    return dict(N=n_nodes, NW=nw, NLOC=nloc,
                NBLK=(nloc + 127) // 128)


def _wrap_idx(arr):
    b = arr.shape[0]
    assert b % 16 == 0
    t = arr.reshape(b // 16, 16).T.copy()
    return np.tile(t, (8, 1)).astype(np.int16)


def _prep_host(x, edge_index, W_src, b_src, W_dst, b_dst, cfg):
    """Pure index reorganization on host -> shared plan + per-core inputs."""
    n, nw, nloc, nblk = cfg["N"], cfg["NW"], cfg["NLOC"], cfg["NBLK"]
    row = np.asarray(edge_index[0], dtype=np.int64)
    col = np.asarray(edge_index[1], dtype=np.int64)

    rp_row = np.zeros(n + 1, dtype=np.int64)
    rp_row[1:] = np.cumsum(np.bincount(row, minlength=n))
    rp_col = np.zeros(n + 1, dtype=np.int64)
    rp_col[1:] = np.cumsum(np.bincount(col, minlength=n))

    def rp_prescale(rp):
        cols = []
        for w in range(nw):
            base = w * WIN
            rows_w = min(WIN, n - base)
            nseg = rows_w // SEG
            for si in range(nseg):
                cols.append(base + si * SEG
                            + np.arange(128)[:, None] * SEGG
                            + np.arange(SEGG)[None, :])
            r0 = base + nseg * SEG
            rem = rows_w - nseg * SEG
            t1 = rem // 128
            if t1:
                cols.append(r0 + np.arange(t1)[None, :] * 128
                            + np.arange(128)[:, None])
            t2 = rem - t1 * 128
            if t2:
                c = r0 + t1 * 128 + np.arange(128)[:, None]
                cols.append(np.where(c < base + rows_w, c, n))
        idx = np.concatenate(cols, axis=1)
        idx = np.minimum(idx, n)
        lo = rp[idx].astype(np.int32)
        hi = rp[np.minimum(idx + 1, n)].astype(np.int32)
        return lo, hi

    rpa_lo, rpa_hi = rp_prescale(rp_row)
    rpb_lo, rpb_hi = rp_prescale(rp_col)

    def rp_local(rp, c):
        idx = (np.arange(nblk)[None, :] * 128 + np.arange(128)[:, None])
        valid = idx < nloc
        idx = np.minimum(c * nloc + idx, n)
        lo = rp[idx]
        hi = rp[np.minimum(idx + 1, n)]
        hi = np.where(valid, hi, lo)
        return lo.astype(np.int32), hi.astype(np.int32)

    def bucket(dest, src):
        """dest-sorted token streams. Returns (plan, g_list, dloc_list)."""
        core = dest // nloc
        pc = []
        cnt = np.zeros((NCORES, nw, nblk), np.int64)
        for c in range(NCORES):
            m = core == c
            d = (dest[m] - c * nloc).astype(np.int64)
            s = src[m].astype(np.int64)
            w = s // WIN
            b = d >> 7
            o = np.lexsort((d, b, w))
            d, s, w, b = d[o], s[o] - w[o] * WIN, w[o], b[o]
            np.add.at(cnt[c], (w, b), 1)
            pc.append((d, s, w, b))
        size_wb = cnt.max(axis=0)                      # [nw, nblk] static
        starts = np.zeros((nw, nblk + 1), np.int64)
        starts[:, 1:] = np.cumsum(size_wb, axis=1)
        wtot = starts[:, -1]
        ntokw = ((wtot + 127) // 128) * 128            # window padded to x128
        win_tok0 = np.zeros(nw, np.int64)
        win_tok0[1:] = np.cumsum(ntokw)[:-1]
        total = int(ntokw.sum())

        # static chunk -> block matmul schedule
        windows = []
        for w in range(nw):
            nch = int(ntokw[w]) // 128
            mms = []                                   # (chunk, block)
            for ci in range(nch):
                lo_t, hi_t = ci * 128, ci * 128 + 128
                for b in range(nblk):
                    if size_wb[w, b] > 0 and starts[w, b] < hi_t \
                            and starts[w, b + 1] > lo_t:
                        mms.append((ci, b))
            calls = [(a, min(CALL, int(ntokw[w]) - a))
                     for a in range(0, int(ntokw[w]), CALL)]
            # psum segment bounds: first/last mm index per block
            seg_first, seg_last = {}, {}
            for j, (ci, b) in enumerate(mms):
                seg_first.setdefault(b, j)
                seg_last[b] = j
            windows.append(dict(tok0=int(win_tok0[w]), ntok=int(ntokw[w]),
                                calls=calls, mms=mms,
                                seg_first=seg_first, seg_last=seg_last))

        g_list, dl_list = [], []
        nmm = sum(len(wd["mms"]) for wd in windows)
        for c in range(NCORES):
            d, s, w, b = pc[c]
            key = w * nblk + b
            gs0 = np.r_[0, np.cumsum(np.bincount(key, minlength=nw * nblk))]
            rank = np.arange(len(d)) - gs0[key]
            pos = win_tok0[w] + starts[w, b] + rank
            g = np.zeros(total, np.int64)
            dl = -np.ones(total, np.int64)
            g[pos] = s
            dl[pos] = d
            # per-mm one-hot dest columns (local id within block or -1)
            cols = np.empty((nmm, 128), np.int16)
            j = 0
            for w2, wd in enumerate(windows):
                dlw = dl[wd["tok0"]:wd["tok0"] + wd["ntok"]].reshape(-1, 128)
                for (ci, b2) in wd["mms"]:
                    r = dlw[ci]
                    cols[j] = np.where((r >= b2 * 128) & (r < (b2 + 1) * 128),
                                       r - b2 * 128, -1).astype(np.int16)
                    j += 1
            g_list.append(_wrap_idx(g.astype(np.int16)))
            dl_list.append(np.ascontiguousarray(cols.T))   # [128, nmm]
        return dict(windows=windows, total=total, nmm=nmm), g_list, dl_list

    plan1, g1, dl1 = bucket(row, col)
    plan2, g2, dl2 = bucket(col, row)

    wsrcT = np.ascontiguousarray(np.asarray(W_src, np.float32).T)
    wdstT = np.ascontiguousarray(np.asarray(W_dst, np.float32).T)
    xf = np.ascontiguousarray(np.asarray(x, np.float32))
    iota = np.tile(np.arange(128, dtype=np.float32)[None, :], (128, 1))

    in_maps = []
    for c in range(NCORES):
        rp1_lo, rp1_hi = rp_local(rp_row, c)
        rp2_lo, rp2_hi = rp_local(rp_col, c)
        in_maps.append({
            "x": xf, "wsrcT": wsrcT, "wdstT": wdstT, "iota": iota,
            "bsrc": np.asarray(b_src, np.float32),
            "bdst": np.asarray(b_dst, np.float32),
            "g1": g1[c], "g2": g2[c],
            "dl1": dl1[c], "dl2": dl2[c],
            "rpa_lo": rpa_lo, "rpa_hi": rpa_hi,
            "rpb_lo": rpb_lo, "rpb_hi": rpb_hi,
            "rp1_lo": rp1_lo, "rp1_hi": rp1_hi,
            "rp2_lo": rp2_lo, "rp2_hi": rp2_hi,
        })
    return in_maps, plan1, plan2


def _build(cfg, plan1, plan2):
    import concourse.tile as tile
    from concourse import bacc, mybir

    dt = mybir.dt
    n, nw = cfg["N"], cfg["NW"]
    nloc, nblk = cfg["NLOC"], cfg["NBLK"]

    nc = bacc.Bacc("TRN2", target_bir_lowering=False, debug=False,
                   num_devices=NCORES)

    x = nc.dram_tensor("x", [n, D], dt.float32, kind="ExternalInput")
    wsrcT = nc.dram_tensor("wsrcT", [D, D], dt.float32, kind="ExternalInput")
    wdstT = nc.dram_tensor("wdstT", [D, D], dt.float32, kind="ExternalInput")
    iota = nc.dram_tensor("iota", [D, D], dt.float32, kind="ExternalInput")
    bsrc = nc.dram_tensor("bsrc", [D], dt.float32, kind="ExternalInput")
    bdst = nc.dram_tensor("bdst", [D], dt.float32, kind="ExternalInput")
    g1 = nc.dram_tensor("g1", [128, plan1["total"] // 16], dt.int16,
                        kind="ExternalInput")
    g2 = nc.dram_tensor("g2", [128, plan2["total"] // 16], dt.int16,
                        kind="ExternalInput")
    dl1 = nc.dram_tensor("dl1", [128, plan1["nmm"]], dt.int16,
                         kind="ExternalInput")
    dl2 = nc.dram_tensor("dl2", [128, plan2["nmm"]], dt.int16,
                         kind="ExternalInput")

    presched = []
    pcols = 0
    for w in range(nw):
        base = w * WIN
        rows_w = min(WIN, n - base)
        nseg = rows_w // SEG
        steps = []
        for si in range(nseg):
            steps.append(("seg", base + si * SEG, SEG, SEGG))
            pcols += SEGG
        r0 = base + nseg * SEG
        rem = rows_w - nseg * SEG
        t1 = rem // 128
        if t1:
            steps.append(("t1", r0, t1 * 128, t1))
            pcols += t1
        t2 = rem - t1 * 128
        if t2:
            steps.append(("t2", r0 + t1 * 128, t2, 1))
            pcols += 1
        presched.append(steps)

    rpa_lo = nc.dram_tensor("rpa_lo", [128, pcols], dt.int32, kind="ExternalInput")
    rpa_hi = nc.dram_tensor("rpa_hi", [128, pcols], dt.int32, kind="ExternalInput")
    rpb_lo = nc.dram_tensor("rpb_lo", [128, pcols], dt.int32, kind="ExternalInput")
    rpb_hi = nc.dram_tensor("rpb_hi", [128, pcols], dt.int32, kind="ExternalInput")
    rp1_lo = nc.dram_tensor("rp1_lo", [128, nblk], dt.int32, kind="ExternalInput")
    rp1_hi = nc.dram_tensor("rp1_hi", [128, nblk], dt.int32, kind="ExternalInput")
    rp2_lo = nc.dram_tensor("rp2_lo", [128, nblk], dt.int32, kind="ExternalInput")
    rp2_hi = nc.dram_tensor("rp2_hi", [128, nblk], dt.int32, kind="ExternalInput")
    out = nc.dram_tensor("out", [nloc, D], dt.float32, kind="ExternalOutput")

    xbw, xaw = [], []
    for w in range(nw):
        rows_w = min(WIN, n - w * WIN)
        xbw.append(nc.dram_tensor(f"xb{w}", [rows_w, D], dt.bfloat16))
        xaw.append(nc.dram_tensor(f"xa{w}", [rows_w, D], dt.bfloat16))

    with tile.TileContext(nc) as tc, ExitStack() as ctx:
        const = ctx.enter_context(tc.tile_pool(name="const", bufs=1))

        wsrcT_sb = const.tile([D, D], dt.float32, tag="wsrc")
        nc.sync.dma_start(wsrcT_sb[:], wsrcT.ap())
        wdstT_sb = const.tile([D, D], dt.float32, tag="wdst")
        nc.sync.dma_start(wdstT_sb[:], wdstT.ap())
        iota_sb = const.tile([D, D], dt.float32, tag="iota")
        nc.sync.dma_start(iota_sb[:], iota.ap())

        brow = const.tile([1, 2 * D], dt.float32, tag="brow")
        nc.sync.dma_start(brow[:, 0:D], bsrc.ap().unsqueeze(0))
        nc.sync.dma_start(brow[:, D:2 * D], bdst.ap().unsqueeze(0))
        bsum = const.tile([1, D], dt.float32, tag="bsum")
        nc.vector.tensor_scalar_mul(bsum[:], brow[:, 0:D], 1.0 - ALPHA)
        bs2 = const.tile([1, D], dt.float32, tag="bs2")
        nc.vector.tensor_scalar_mul(bs2[:], brow[:, D:2 * D], ALPHA)
        nc.vector.tensor_add(bsum[:], bsum[:], bs2[:])
        bias_bc = const.tile([D, D], dt.float32, tag="biasbc")
        nc.gpsimd.partition_broadcast(bias_bc[:], bsum[:])

        g1_sb = const.tile([128, plan1["total"] // 16], dt.int16, tag="g1")
        nc.sync.dma_start(g1_sb[:], g1.ap())
        g2_sb = const.tile([128, plan2["total"] // 16], dt.int16, tag="g2")
        nc.sync.dma_start(g2_sb[:], g2.ap())

        dlf1 = const.tile([128, plan1["nmm"]], dt.float32, tag="dlf1")
        dlf2 = const.tile([128, plan2["nmm"]], dt.float32, tag="dlf2")

        def invsqrt_chain(pool, lo_ap, hi_ap, cols, tag, scale=None,
                          res_pool=None):
            res_pool = res_pool or pool
            lo_t = pool.tile([128, cols], dt.int32, tag=tag + "lo")
            nc.sync.dma_start(lo_t[:], lo_ap)
            hi_t = pool.tile([128, cols], dt.int32, tag=tag + "hi")
            nc.sync.dma_start(hi_t[:], hi_ap)
            deg_i = pool.tile([128, cols], dt.int32, tag=tag + "di")
            nc.vector.tensor_sub(deg_i[:], hi_t[:], lo_t[:])
            deg_f = pool.tile([128, cols], dt.float32, tag=tag + "df")
            nc.vector.tensor_copy(deg_f[:], deg_i[:])
            mask = pool.tile([128, cols], dt.float32, tag=tag + "mk")
            mul = scale if scale is not None else 1.0
            nc.vector.tensor_scalar(mask[:], deg_f[:], 1.0, mul,
                                    mybir.AluOpType.min, mybir.AluOpType.mult)
            dmax = pool.tile([128, cols], dt.float32, tag=tag + "dm")
            nc.vector.tensor_scalar_max(dmax[:], deg_f[:], 1.0)
            rec = pool.tile([128, cols], dt.float32, tag=tag + "rc")
            nc.vector.reciprocal(rec[:], dmax[:])
            sq = pool.tile([128, cols], dt.float32, tag=tag + "sq")
            nc.scalar.sqrt(sq[:], rec[:])
            res = res_pool.tile([128, cols], dt.float32, tag=tag + "rs")
            nc.vector.tensor_mul(res[:], sq[:], mask[:])
            return res

        with tc.tile_pool(name="chainscratch", bufs=1) as csp:
            a_vec = invsqrt_chain(csp, rp1_lo.ap(), rp1_hi.ap(), nblk, "av",
                                  scale=1.0 - ALPHA, res_pool=const)
            b_vec = invsqrt_chain(csp, rp2_lo.ap(), rp2_hi.ap(), nblk, "bv",
                                  scale=ALPHA, res_pool=const)
            b_full = invsqrt_chain(csp, rpb_lo.ap(), rpb_hi.ap(), pcols,
                                   "bf", res_pool=const)
            a_full = invsqrt_chain(csp, rpa_lo.ap(), rpa_hi.ap(), pcols,
                                   "af", res_pool=const)
            di1 = csp.tile([128, plan1["nmm"]], dt.int16, tag="di1")
            nc.sync.dma_start(di1[:], dl1.ap())
            nc.vector.tensor_copy(dlf1[:], di1[:])
            di2 = csp.tile([128, plan2["nmm"]], dt.int16, tag="di2")
            nc.sync.dma_start(di2[:], dl2.ap())
            nc.vector.tensor_copy(dlf2[:], di2[:])

        # SBUF accumulators [feat x dest], one per direction
        agg1_sb = const.tile([128, nblk * 128], dt.float32, tag="agg1")
        agg2_sb = const.tile([128, nblk * 128], dt.float32, tag="agg2")
        for agg in (agg1_sb, agg2_sb):
            off = 0
            while off < nblk * 128:
                csz = min(4096, nblk * 128 - off)
                nc.vector.memset(agg[:, off:off + csz], 0.0)
                off += csz

        gpool = ctx.enter_context(tc.tile_pool(name="gat", bufs=4))
        spool = ctx.enter_context(tc.tile_pool(name="sb", bufs=4))
        epsum = ctx.enter_context(tc.tile_pool(name="eps", bufs=4,
                                               space="PSUM"))

        def prescale_window(pp, w, ccur):
            for kind_, r0, nrows, ncols in presched[w]:
                base = w * WIN
                cs = slice(ccur, ccur + ncols)
                ccur += ncols
                if kind_ == "t2":
                    xt = pp.tile([nrows, D], dt.float32, tag="pxt2")
                    nc.sync.dma_start(xt[:], x.ap()[r0:r0 + nrows, :])
                    for dest, sv in ((xbw[w], b_full), (xaw[w], a_full)):
                        ot = pp.tile([nrows, D], dt.bfloat16, tag="pot2")
                        nc.scalar.mul(ot[:], xt[:], sv[0:nrows, cs])
                        nc.sync.dma_start(
                            dest.ap()[r0 - base:r0 - base + nrows, :], ot[:])
                    continue
                wrap = "(p g) d -> p g d" if kind_ == "seg" \
                    else "(g p) d -> p g d"
                xs = x.ap()[r0:r0 + nrows, :].rearrange(wrap, p=128)
                xt = pp.tile([128, ncols, D], dt.float32, tag="pxt")
                nc.sync.dma_start(xt[:], xs)
                for dest, sv in ((xbw[w], b_full), (xaw[w], a_full)):
                    ex = pp.tile([128, ncols, D], dt.bfloat16, tag="pex")
                    nc.vector.tensor_tensor(
                        ex[:], sv[:, cs].unsqueeze(2).to_broadcast(
                            [128, ncols, D]),
                        xt[:], mybir.AluOpType.mult)
                    dv = dest.ap()[r0 - base:r0 - base + nrows, :] \
                        .rearrange(wrap, p=128)
                    nc.sync.dma_start(dv, ex[:])
            return ccur

        def edge_window(plan, w, g_sb, dlf, srcw, agg_sb, mm0, dtag):
            wd = plan["windows"][w]
            xs = srcw[w].ap()
            tiles = {}
            for (a, ln) in wd["calls"]:
                xt = gpool.tile([128, CALL // 128, D], dt.bfloat16,
                                tag="xt" + dtag)
                o = wd["tok0"] + a
                gi = g_sb[:, o // 16:(o + ln) // 16]
                nc.gpsimd.dma_gather(xt[:, 0:ln // 128, :], xs, gi, ln, ln, D)
                for g in range(ln // 128):
                    tiles[a // 128 + g] = (xt, g)
            mms = wd["mms"]
            # batched one-hot builds (8 consecutive mm columns per op)
            sb_tiles = []
            for j0 in range(0, len(mms), 8):
                jn = min(8, len(mms) - j0)
                st = spool.tile([128, 8, D], dt.bfloat16, tag="st")
                nc.vector.tensor_tensor(
                    st[:, 0:jn, :],
                    dlf[:, mm0 + j0:mm0 + j0 + jn].unsqueeze(2)
                    .to_broadcast([128, jn, D]),
                    iota_sb[:].unsqueeze(1).to_broadcast([128, jn, D]),
                    mybir.AluOpType.is_equal)
                sb_tiles.append(st)
            active = {}
            for j, (ci, b) in enumerate(mms):
                xt, g = tiles[ci]
                st = sb_tiles[j // 8]
                if b not in active:
                    active[b] = epsum.tile([128, D], dt.float32, tag="ep",
                                           name="ep")
                ps = active[b]
                nc.tensor.matmul(ps[:], lhsT=xt[:, g, :], rhs=st[:, j % 8, :],
                                 start=(j == wd["seg_first"][b]),
                                 stop=(j == wd["seg_last"][b]))
                if j == wd["seg_last"][b]:
                    nc.vector.tensor_add(
                        agg_sb[:, b * 128:(b + 1) * 128],
                        agg_sb[:, b * 128:(b + 1) * 128], ps[:])
                    del active[b]
            return mm0 + len(mms)

        # interleave: pre0, pre1, edges0, pre2, edges1, pre3, edges2, edges3
        with tc.tile_pool(name="prescale", bufs=2) as pp:
            ccur = [0]
            mm1 = [0]
            mm2 = [0]

            def do_pre(w):
                ccur[0] = prescale_window(pp, w, ccur[0])

            def do_edges(w):
                mm1[0] = edge_window(plan1, w, g1_sb, dlf1, xbw, agg1_sb,
                                     mm1[0], "1")
                mm2[0] = edge_window(plan2, w, g2_sb, dlf2, xaw, agg2_sb,
                                     mm2[0], "2")

            do_pre(0)
            if nw > 1:
                do_pre(1)
            do_edges(0)
            for w in range(2, nw):
                do_pre(w)
                do_edges(w - 1)
            for w in range(max(1, nw - 1), nw):
                do_edges(w)

        # final: per block out = ao.agg1@WsrcT + bi.agg2@WdstT + bias
        with tc.tile_pool(name="fin", bufs=3) as fp, \
             tc.tile_pool(name="fps", bufs=2, space="PSUM") as fps:
            for k in range(nblk):
                ks = slice(k * 128, (k + 1) * 128)
                p1 = fps.tile([128, D], dt.float32, tag="p1")
                nc.tensor.matmul(p1[:], lhsT=agg1_sb[:, ks], rhs=wsrcT_sb[:],
                                 start=True, stop=True)
                p2 = fps.tile([128, D], dt.float32, tag="p2")
                nc.tensor.matmul(p2[:], lhsT=agg2_sb[:, ks], rhs=wdstT_sb[:],
                                 start=True, stop=True)
                o1 = fp.tile([128, D], dt.float32, tag="o1")
                nc.scalar.mul(o1[:], p1[:], a_vec[:, k:k + 1])
                o2 = fp.tile([128, D], dt.float32, tag="o2")
                nc.scalar.mul(o2[:], p2[:], b_vec[:, k:k + 1])
                fin = fp.tile([128, D], dt.float32, tag="fin")
                nc.vector.tensor_add(fin[:], o1[:], o2[:])
                nc.vector.tensor_add(fin[:], fin[:], bias_bc[:])
                rows = min(128, nloc - k * 128)
                nc.sync.dma_start(out.ap()[k * 128:k * 128 + rows, :],
                                  fin[0:rows, :])

    nc.compile()
    return nc


def _install_ntff_shim():
    """This image's antenv lacks axon_hooks; inject it so trace=True works."""
    import sys
    import types
    try:
        from antenv import axon_hooks  # noqa: F401
        return
    except ImportError:
        pass
    try:
        import antenv
        from trn_agent_boot.trn_boot import _ntff_profile_via_ctypes
        mod = types.ModuleType("antenv.axon_hooks")
        holder = [None]
        mod.set_axon_ntff_profile_hook = lambda h: holder.__setitem__(0, h)
        mod.get_axon_ntff_profile_hook = lambda: holder[0]
        sys.modules["antenv.axon_hooks"] = mod
        antenv.axon_hooks = mod
        mod.set_axon_ntff_profile_hook(
            _ntff_profile_via_ctypes("/opt/axon/libaxon_pjrt.so"))
    except Exception as e:  # profiling is best-effort
        print("ntff shim failed:", e)


def _run(nc, in_maps, trace=False):
    from concourse.bass_utils import run_bass_kernel_spmd
    kw = {}
    if trace:
        _install_ntff_shim()
        kw = dict(trace=True, trace_cores=list(range(NCORES)))
    return run_bass_kernel_spmd(nc, in_maps, list(range(NCORES)), **kw)


def kernel(x, edge_index, W_src, b_src, W_dst, b_dst, _trace=False,
           _return_result=False):
    cfg = _cfg_for(x.shape[0])
    in_maps, plan1, plan2 = _prep_host(x, edge_index, W_src, b_src, W_dst,
                                       b_dst, cfg)
    nc = _build(cfg, plan1, plan2)
    res = _run(nc, in_maps, trace=_trace)
    out = np.concatenate([res.results[c]["out"] for c in range(NCORES)],
                         axis=0)
    if _return_result:
        return out, res
    return out


# revision 9
# speedup vs baseline: 2.3982x; 1.0534x over previous
"""DirGCNConv on 8 Trainium2 NeuronCores via Bass/Tile (v2: scatter-free).

out = (1-a)*(Dout^-1/2 A Din^-1/2 x) @ Wsrc.T + a*(Din^-1/2 A.T Dout^-1/2 x) @ Wdst.T + bias

Per-edge weight separates: w[e] = ao[row[e]] * bi[col[e]], so each direction
is agg[dest] = Sum_{edges} prescaled_x[src], then a per-dest scale + matmul.

v2 strategy (vs v1 gather+scatter-add): edges are sorted by *destination*;
gathered source rows (bf16, dma_gather) are reduced per dest block with
one-hot segment matmuls on the Tensor engine accumulating in PSUM, then
folded into an SBUF-resident [feat x dest] accumulator. This removes all
dma_scatter_add calls — the GpSimd descriptor-generation engine (the
bottleneck) only runs gathers.

SPMD: one program for all 8 cores, so the chunk/matmul schedule is static:
each (window, dest-block) segment is padded to the max token count over
cores. Per-core data (gather indices, one-hot dest columns) differs only in
tensor contents.
"""

import os

import numpy as np
from contextlib import ExitStack

os.environ.setdefault("NEURON_RT_RESET_CORES", "1")

N = 100000
E = 600000
D = 128
NCORES = 8
ALPHA = 0.5

WIN = 25000          # gather source window rows (int16 range allows 32768)
CALL = 1024          # max tokens per dma_gather call
SEG = 2048           # prescale segment rows
SEGG = SEG // 128


def _cfg_for(n_nodes):
    nw = (n_nodes + WIN - 1) // WIN
    nloc = n_nodes // NCORES
    return dict(N=n_nodes, NW=nw, NLOC=nloc,
                NBLK=(nloc + 127) // 128)


def _wrap_idx(arr):
    b = arr.shape[0]
    assert b % 16 == 0
    t = arr.reshape(b // 16, 16).T.copy()
    return np.tile(t, (8, 1)).astype(np.int16)


def _prep_host(x, edge_index, W_src, b_src, W_dst, b_dst, cfg):
    """Pure index reorganization on host -> shared plan + per-core inputs."""
    n, nw, nloc, nblk = cfg["N"], cfg["NW"], cfg["NLOC"], cfg["NBLK"]
    row = np.asarray(edge_index[0], dtype=np.int64)
    col = np.asarray(edge_index[1], dtype=np.int64)

    rp_row = np.zeros(n + 1, dtype=np.int64)
    rp_row[1:] = np.cumsum(np.bincount(row, minlength=n))
    rp_col = np.zeros(n + 1, dtype=np.int64)
    rp_col[1:] = np.cumsum(np.bincount(col, minlength=n))

    def rp_prescale(rp):
        cols = []
        for w in range(nw):
            base = w * WIN
            rows_w = min(WIN, n - base)
            nseg = rows_w // SEG
            for si in range(nseg):
                cols.append(base + si * SEG
                            + np.arange(128)[:, None] * SEGG
                            + np.arange(SEGG)[None, :])
            r0 = base + nseg * SEG
            rem = rows_w - nseg * SEG
            t1 = rem // 128
            if t1:
                cols.append(r0 + np.arange(t1)[None, :] * 128
                            + np.arange(128)[:, None])
            t2 = rem - t1 * 128
            if t2:
                c = r0 + t1 * 128 + np.arange(128)[:, None]
                cols.append(np.where(c < base + rows_w, c, n))
        idx = np.concatenate(cols, axis=1)
        idx = np.minimum(idx, n)
        lo = rp[idx].astype(np.int32)
        hi = rp[np.minimum(idx + 1, n)].astype(np.int32)
        return lo, hi

    rpa_lo, rpa_hi = rp_prescale(rp_row)
    rpb_lo, rpb_hi = rp_prescale(rp_col)

    def rp_local(rp, c):
        idx = (np.arange(nblk)[None, :] * 128 + np.arange(128)[:, None])
        valid = idx < nloc
        idx = np.minimum(c * nloc + idx, n)
        lo = rp[idx]
        hi = rp[np.minimum(idx + 1, n)]
        hi = np.where(valid, hi, lo)
        return lo.astype(np.int32), hi.astype(np.int32)

    def bucket(dest, src):
        """dest-sorted token streams. Returns (plan, g_list, dloc_list)."""
        core = dest // nloc
        pc = []
        cnt = np.zeros((NCORES, nw, nblk), np.int64)
        for c in range(NCORES):
            m = core == c
            d = (dest[m] - c * nloc).astype(np.int64)
            s = src[m].astype(np.int64)
            w = s // WIN
            b = d >> 7
            o = np.lexsort((d, b, w))
            d, s, w, b = d[o], s[o] - w[o] * WIN, w[o], b[o]
            np.add.at(cnt[c], (w, b), 1)
            pc.append((d, s, w, b))
        size_wb = cnt.max(axis=0)                      # [nw, nblk] static
        starts = np.zeros((nw, nblk + 1), np.int64)
        starts[:, 1:] = np.cumsum(size_wb, axis=1)
        wtot = starts[:, -1]
        ntokw = ((wtot + 127) // 128) * 128            # window padded to x128
        win_tok0 = np.zeros(nw, np.int64)
        win_tok0[1:] = np.cumsum(ntokw)[:-1]
        total = int(ntokw.sum())

        # static chunk -> block matmul schedule
        windows = []
        for w in range(nw):
            nch = int(ntokw[w]) // 128
            mms = []                                   # (chunk, block)
            for ci in range(nch):
                lo_t, hi_t = ci * 128, ci * 128 + 128
                for b in range(nblk):
                    if size_wb[w, b] > 0 and starts[w, b] < hi_t \
                            and starts[w, b + 1] > lo_t:
                        mms.append((ci, b))
            calls = [(a, min(CALL, int(ntokw[w]) - a))
                     for a in range(0, int(ntokw[w]), CALL)]
            # psum segment bounds: first/last mm index per block
            seg_first, seg_last = {}, {}
            for j, (ci, b) in enumerate(mms):
                seg_first.setdefault(b, j)
                seg_last[b] = j
            windows.append(dict(tok0=int(win_tok0[w]), ntok=int(ntokw[w]),
                                calls=calls, mms=mms,
                                seg_first=seg_first, seg_last=seg_last))

        g_list, dl_list = [], []
        nmm = sum(len(wd["mms"]) for wd in windows)
        for c in range(NCORES):
            d, s, w, b = pc[c]
            key = w * nblk + b
            gs0 = np.r_[0, np.cumsum(np.bincount(key, minlength=nw * nblk))]
            rank = np.arange(len(d)) - gs0[key]
            pos = win_tok0[w] + starts[w, b] + rank
            g = np.zeros(total, np.int64)
            dl = -np.ones(total, np.int64)
            g[pos] = s
            dl[pos] = d
            # per-mm one-hot dest columns (local id within block or -1)
            cols = np.empty((nmm, 128), np.int16)
            j = 0
            for w2, wd in enumerate(windows):
                dlw = dl[wd["tok0"]:wd["tok0"] + wd["ntok"]].reshape(-1, 128)
                for (ci, b2) in wd["mms"]:
                    r = dlw[ci]
                    cols[j] = np.where((r >= b2 * 128) & (r < (b2 + 1) * 128),
                                       r - b2 * 128, -1).astype(np.int16)
                    j += 1
            g_list.append(_wrap_idx(g.astype(np.int16)))
            dl_list.append(np.ascontiguousarray(cols.T))   # [128, nmm]
        return dict(windows=windows, total=total, nmm=nmm), g_list, dl_list

    plan1, g1, dl1 = bucket(row, col)
    plan2, g2, dl2 = bucket(col, row)

    wsrcT = np.ascontiguousarray(np.asarray(W_src, np.float32).T)
    wdstT = np.ascontiguousarray(np.asarray(W_dst, np.float32).T)
    xf = np.ascontiguousarray(np.asarray(x, np.float32))
    iota = np.tile(np.arange(128, dtype=np.float32)[None, :], (128, 1))

    in_maps = []
    for c in range(NCORES):
        rp1_lo, rp1_hi = rp_local(rp_row, c)
        rp2_lo, rp2_hi = rp_local(rp_col, c)
        in_maps.append({
            "x": xf, "wsrcT": wsrcT, "wdstT": wdstT, "iota": iota,
            "bsrc": np.asarray(b_src, np.float32),
            "bdst": np.asarray(b_dst, np.float32),
            "g1": g1[c], "g2": g2[c],
            "dl1": dl1[c], "dl2": dl2[c],
            "rpa_lo": rpa_lo, "rpa_hi": rpa_hi,
            "rpb_lo": rpb_lo, "rpb_hi": rpb_hi,
            "rp1_lo": rp1_lo, "rp1_hi": rp1_hi,
            "rp2_lo": rp2_lo, "rp2_hi": rp2_hi,
        })
    return in_maps, plan1, plan2


def _build(cfg, plan1, plan2):
    import concourse.tile as tile
    from concourse import bacc, mybir

    dt = mybir.dt
    n, nw = cfg["N"], cfg["NW"]
    nloc, nblk = cfg["NLOC"], cfg["NBLK"]

    nc = bacc.Bacc("TRN2", target_bir_lowering=False, debug=False,
                   num_devices=NCORES)

    x = nc.dram_tensor("x", [n, D], dt.float32, kind="ExternalInput")
    wsrcT = nc.dram_tensor("wsrcT", [D, D], dt.float32, kind="ExternalInput")
    wdstT = nc.dram_tensor("wdstT", [D, D], dt.float32, kind="ExternalInput")
    iota = nc.dram_tensor("iota", [D, D], dt.float32, kind="ExternalInput")
    bsrc = nc.dram_tensor("bsrc", [D], dt.float32, kind="ExternalInput")
    bdst = nc.dram_tensor("bdst", [D], dt.float32, kind="ExternalInput")
    g1 = nc.dram_tensor("g1", [128, plan1["total"] // 16], dt.int16,
                        kind="ExternalInput")
    g2 = nc.dram_tensor("g2", [128, plan2["total"] // 16], dt.int16,
                        kind="ExternalInput")
    dl1 = nc.dram_tensor("dl1", [128, plan1["nmm"]], dt.int16,
                         kind="ExternalInput")
    dl2 = nc.dram_tensor("dl2", [128, plan2["nmm"]], dt.int16,
                         kind="ExternalInput")

    presched = []
    pcols = 0
    for w in range(nw):
        base = w * WIN
        rows_w = min(WIN, n - base)
        nseg = rows_w // SEG
        steps = []
        for si in range(nseg):
            steps.append(("seg", base + si * SEG, SEG, SEGG))
            pcols += SEGG
        r0 = base + nseg * SEG
        rem = rows_w - nseg * SEG
        t1 = rem // 128
        if t1:
            steps.append(("t1", r0, t1 * 128, t1))
            pcols += t1
        t2 = rem - t1 * 128
        if t2:
            steps.append(("t2", r0 + t1 * 128, t2, 1))
            pcols += 1
        presched.append(steps)

    rpa_lo = nc.dram_tensor("rpa_lo", [128, pcols], dt.int32, kind="ExternalInput")
    rpa_hi = nc.dram_tensor("rpa_hi", [128, pcols], dt.int32, kind="ExternalInput")
    rpb_lo = nc.dram_tensor("rpb_lo", [128, pcols], dt.int32, kind="ExternalInput")
    rpb_hi = nc.dram_tensor("rpb_hi", [128, pcols], dt.int32, kind="ExternalInput")
    rp1_lo = nc.dram_tensor("rp1_lo", [128, nblk], dt.int32, kind="ExternalInput")
    rp1_hi = nc.dram_tensor("rp1_hi", [128, nblk], dt.int32, kind="ExternalInput")
    rp2_lo = nc.dram_tensor("rp2_lo", [128, nblk], dt.int32, kind="ExternalInput")
    rp2_hi = nc.dram_tensor("rp2_hi", [128, nblk], dt.int32, kind="ExternalInput")
    out = nc.dram_tensor("out", [nloc, D], dt.float32, kind="ExternalOutput")

    xbw, xaw = [], []
    for w in range(nw):
        rows_w = min(WIN, n - w * WIN)
        xbw.append(nc.dram_tensor(f"xb{w}", [rows_w, D], dt.bfloat16))
        xaw.append(nc.dram_tensor(f"xa{w}", [rows_w, D], dt.bfloat16))

    with tile.TileContext(nc) as tc, ExitStack() as ctx:
        const = ctx.enter_context(tc.tile_pool(name="const", bufs=1))

        wsrcT_sb = const.tile([D, D], dt.float32, tag="wsrc")
        nc.sync.dma_start(wsrcT_sb[:], wsrcT.ap())
        wdstT_sb = const.tile([D, D], dt.float32, tag="wdst")
        nc.sync.dma_start(wdstT_sb[:], wdstT.ap())
        iota_sb = const.tile([D, D], dt.float32, tag="iota")
        nc.sync.dma_start(iota_sb[:], iota.ap())

        brow = const.tile([1, 2 * D], dt.float32, tag="brow")
        nc.sync.dma_start(brow[:, 0:D], bsrc.ap().unsqueeze(0))
        nc.sync.dma_start(brow[:, D:2 * D], bdst.ap().unsqueeze(0))
        bsum = const.tile([1, D], dt.float32, tag="bsum")
        nc.vector.tensor_scalar_mul(bsum[:], brow[:, 0:D], 1.0 - ALPHA)
        bs2 = const.tile([1, D], dt.float32, tag="bs2")
        nc.vector.tensor_scalar_mul(bs2[:], brow[:, D:2 * D], ALPHA)
        nc.vector.tensor_add(bsum[:], bsum[:], bs2[:])
        bias_bc = const.tile([D, D], dt.float32, tag="biasbc")
        nc.gpsimd.partition_broadcast(bias_bc[:], bsum[:])

        g1_sb = const.tile([128, plan1["total"] // 16], dt.int16, tag="g1")
        nc.sync.dma_start(g1_sb[:], g1.ap())
        g2_sb = const.tile([128, plan2["total"] // 16], dt.int16, tag="g2")
        nc.sync.dma_start(g2_sb[:], g2.ap())

        dlf1 = const.tile([128, plan1["nmm"]], dt.float32, tag="dlf1")
        dlf2 = const.tile([128, plan2["nmm"]], dt.float32, tag="dlf2")

        def invsqrt_chain(pool, lo_ap, hi_ap, cols, tag, scale=None,
                          res_pool=None):
            res_pool = res_pool or pool
            lo_t = pool.tile([128, cols], dt.int32, tag=tag + "lo")
            nc.sync.dma_start(lo_t[:], lo_ap)
            hi_t = pool.tile([128, cols], dt.int32, tag=tag + "hi")
            nc.sync.dma_start(hi_t[:], hi_ap)
            deg_i = pool.tile([128, cols], dt.int32, tag=tag + "di")
            nc.vector.tensor_sub(deg_i[:], hi_t[:], lo_t[:])
            deg_f = pool.tile([128, cols], dt.float32, tag=tag + "df")
            nc.vector.tensor_copy(deg_f[:], deg_i[:])
            mask = pool.tile([128, cols], dt.float32, tag=tag + "mk")
            mul = scale if scale is not None else 1.0
            nc.vector.tensor_scalar(mask[:], deg_f[:], 1.0, mul,
                                    mybir.AluOpType.min, mybir.AluOpType.mult)
            dmax = pool.tile([128, cols], dt.float32, tag=tag + "dm")
            nc.vector.tensor_scalar_max(dmax[:], deg_f[:], 1.0)
            rec = pool.tile([128, cols], dt.float32, tag=tag + "rc")
            nc.vector.reciprocal(rec[:], dmax[:])
            sq = pool.tile([128, cols], dt.float32, tag=tag + "sq")
            nc.scalar.sqrt(sq[:], rec[:])
            res = res_pool.tile([128, cols], dt.float32, tag=tag + "rs")
            nc.vector.tensor_mul(res[:], sq[:], mask[:])
            return res

        with tc.tile_pool(name="chainscratch", bufs=1) as csp:
            a_vec = invsqrt_chain(csp, rp1_lo.ap(), rp1_hi.ap(), nblk, "av",
                                  scale=1.0 - ALPHA, res_pool=const)
            b_vec = invsqrt_chain(csp, rp2_lo.ap(), rp2_hi.ap(), nblk, "bv",
                                  scale=ALPHA, res_pool=const)
            b_full = invsqrt_chain(csp, rpb_lo.ap(), rpb_hi.ap(), pcols,
                                   "bf", res_pool=const)
            a_full = invsqrt_chain(csp, rpa_lo.ap(), rpa_hi.ap(), pcols,
                                   "af", res_pool=const)
            di1 = csp.tile([128, plan1["nmm"]], dt.int16, tag="di1")
            nc.sync.dma_start(di1[:], dl1.ap())
            nc.vector.tensor_copy(dlf1[:], di1[:])
            di2 = csp.tile([128, plan2["nmm"]], dt.int16, tag="di2")
            nc.sync.dma_start(di2[:], dl2.ap())
            nc.vector.tensor_copy(dlf2[:], di2[:])

        # SBUF accumulators [feat x dest], one per direction
        agg1_sb = const.tile([128, nblk * 128], dt.float32, tag="agg1")
        agg2_sb = const.tile([128, nblk * 128], dt.float32, tag="agg2")
        for agg in (agg1_sb, agg2_sb):
            off = 0
            while off < nblk * 128:
                csz = min(4096, nblk * 128 - off)
                nc.vector.memset(agg[:, off:off + csz], 0.0)
                off += csz

        gpool = ctx.enter_context(tc.tile_pool(name="gat", bufs=4))
        spool = ctx.enter_context(tc.tile_pool(name="sb", bufs=4))
        epsum = ctx.enter_context(tc.tile_pool(name="eps", bufs=4,
                                               space="PSUM"))

        def prescale_window(pp, w, which, ccur):
            dest, sv = (xbw[w], b_full) if which == "b" else (xaw[w], a_full)
            for kind_, r0, nrows, ncols in presched[w]:
                base = w * WIN
                cs = slice(ccur, ccur + ncols)
                ccur += ncols
                if kind_ == "t2":
                    xt = pp.tile([nrows, D], dt.float32, tag="pxt2")
                    nc.sync.dma_start(xt[:], x.ap()[r0:r0 + nrows, :])
                    ot = pp.tile([nrows, D], dt.bfloat16, tag="pot2")
                    nc.scalar.mul(ot[:], xt[:], sv[0:nrows, cs])
                    nc.sync.dma_start(
                        dest.ap()[r0 - base:r0 - base + nrows, :], ot[:])
                    continue
                wrap = "(p g) d -> p g d" if kind_ == "seg" \
                    else "(g p) d -> p g d"
                xs = x.ap()[r0:r0 + nrows, :].rearrange(wrap, p=128)
                xt = pp.tile([128, ncols, D], dt.float32, tag="pxt")
                nc.sync.dma_start(xt[:], xs)
                ex = pp.tile([128, ncols, D], dt.bfloat16, tag="pex")
                nc.vector.tensor_tensor(
                    ex[:], sv[:, cs].unsqueeze(2).to_broadcast(
                        [128, ncols, D]),
                    xt[:], mybir.AluOpType.mult)
                dv = dest.ap()[r0 - base:r0 - base + nrows, :] \
                    .rearrange(wrap, p=128)
                nc.sync.dma_start(dv, ex[:])
            return ccur

        def edge_window(plan, w, g_sb, dlf, srcw, agg_sb, mm0, dtag,
                        final_cb=None):
            wd = plan["windows"][w]
            xs = srcw[w].ap()
            tiles = {}
            for (a, ln) in wd["calls"]:
                xt = gpool.tile([128, CALL // 128, D], dt.bfloat16,
                                tag="xt" + dtag)
                o = wd["tok0"] + a
                gi = g_sb[:, o // 16:(o + ln) // 16]
                nc.gpsimd.dma_gather(xt[:, 0:ln // 128, :], xs, gi, ln, ln, D)
                for g in range(ln // 128):
                    tiles[a // 128 + g] = (xt, g)
            mms = wd["mms"]
            # batched one-hot builds (8 consecutive mm columns per op)
            sb_tiles = []
            for j0 in range(0, len(mms), 8):
                jn = min(8, len(mms) - j0)
                st = spool.tile([128, 8, D], dt.bfloat16, tag="st")
                nc.vector.tensor_tensor(
                    st[:, 0:jn, :],
                    dlf[:, mm0 + j0:mm0 + j0 + jn].unsqueeze(2)
                    .to_broadcast([128, jn, D]),
                    iota_sb[:].unsqueeze(1).to_broadcast([128, jn, D]),
                    mybir.AluOpType.is_equal)
                sb_tiles.append(st)
            active = {}
            for j, (ci, b) in enumerate(mms):
                xt, g = tiles[ci]
                st = sb_tiles[j // 8]
                if b not in active:
                    active[b] = epsum.tile([128, D], dt.float32, tag="ep",
                                           name="ep")
                ps = active[b]
                nc.tensor.matmul(ps[:], lhsT=xt[:, g, :], rhs=st[:, j % 8, :],
                                 start=(j == wd["seg_first"][b]),
                                 stop=(j == wd["seg_last"][b]))
                if j == wd["seg_last"][b]:
                    nc.vector.tensor_add(
                        agg_sb[:, b * 128:(b + 1) * 128],
                        agg_sb[:, b * 128:(b + 1) * 128], ps[:])
                    del active[b]
                    if final_cb is not None:
                        final_cb(b)
            return mm0 + len(mms)

        # final per-block output, emitted as soon as a block's accumulators
        # are complete (interleaved into the last window's dir-2 stream)
        fp = ctx.enter_context(tc.tile_pool(name="fin", bufs=3))
        fps = ctx.enter_context(tc.tile_pool(name="fps", bufs=2,
                                             space="PSUM"))

        def final_block(k):
            ks = slice(k * 128, (k + 1) * 128)
            p1 = fps.tile([128, D], dt.float32, tag="p1", name="p1")
            nc.tensor.matmul(p1[:], lhsT=agg1_sb[:, ks], rhs=wsrcT_sb[:],
                             start=True, stop=True)
            p2 = fps.tile([128, D], dt.float32, tag="p2", name="p2")
            nc.tensor.matmul(p2[:], lhsT=agg2_sb[:, ks], rhs=wdstT_sb[:],
                             start=True, stop=True)
            o1 = fp.tile([128, D], dt.float32, tag="o1", name="o1")
            nc.scalar.mul(o1[:], p1[:], a_vec[:, k:k + 1])
            o2 = fp.tile([128, D], dt.float32, tag="o2", name="o2")
            nc.scalar.mul(o2[:], p2[:], b_vec[:, k:k + 1])
            fin = fp.tile([128, D], dt.float32, tag="fin", name="fin")
            nc.vector.tensor_add(fin[:], o1[:], o2[:])
            nc.vector.tensor_add(fin[:], fin[:], bias_bc[:])
            rows = min(128, nloc - k * 128)
            nc.sync.dma_start(out.ap()[k * 128:k * 128 + rows, :],
                              fin[0:rows, :])

        emitted = set()

        def final_cb(b):
            if b not in emitted:
                emitted.add(b)
                final_block(b)

        # interleave: xb prescale feeds dir-1 gathers, xa feeds dir-2;
        # window w+1 prescale is emitted before window w's edge stream
        with tc.tile_pool(name="prescale", bufs=2) as pp:
            ccb, cca, mm1, mm2 = [0], [0], [0], [0]

            def pre_b(w):
                ccb[0] = prescale_window(pp, w, "b", ccb[0])

            def pre_a(w):
                cca[0] = prescale_window(pp, w, "a", cca[0])

            def e1(w, cb=None):
                mm1[0] = edge_window(plan1, w, g1_sb, dlf1, xbw, agg1_sb,
                                     mm1[0], "1", cb)

            def e2(w, cb=None):
                mm2[0] = edge_window(plan2, w, g2_sb, dlf2, xaw, agg2_sb,
                                     mm2[0], "2", cb)

            pre_b(0)
            pre_a(0)
            if nw > 1:
                pre_b(1)
            e1(0)
            if nw > 1:
                pre_a(1)
            e2(0)
            for w in range(1, nw - 1):
                pre_b(w + 1)
                e1(w)
                pre_a(w + 1)
                e2(w)
            if nw > 1:
                e1(nw - 1)
                e2(nw - 1, final_cb)
        for k in range(nblk):
            if k not in emitted:
                emitted.add(k)
                final_block(k)

    nc.compile()
    return nc


def _install_ntff_shim():
    """This image's antenv lacks axon_hooks; inject it so trace=True works."""
    import sys
    import types
    try:
        from antenv import axon_hooks  # noqa: F401
        return
    except ImportError:
        pass
    try:
        import antenv
        from trn_agent_boot.trn_boot import _ntff_profile_via_ctypes
        mod = types.ModuleType("antenv.axon_hooks")
        holder = [None]
        mod.set_axon_ntff_profile_hook = lambda h: holder.__setitem__(0, h)
        mod.get_axon_ntff_profile_hook = lambda: holder[0]
        sys.modules["antenv.axon_hooks"] = mod
        antenv.axon_hooks = mod
        mod.set_axon_ntff_profile_hook(
            _ntff_profile_via_ctypes("/opt/axon/libaxon_pjrt.so"))
    except Exception as e:  # profiling is best-effort
        print("ntff shim failed:", e)


def _run(nc, in_maps, trace=False):
    from concourse.bass_utils import run_bass_kernel_spmd
    kw = {}
    if trace:
        _install_ntff_shim()
        kw = dict(trace=True, trace_cores=list(range(NCORES)))
    return run_bass_kernel_spmd(nc, in_maps, list(range(NCORES)), **kw)


def kernel(x, edge_index, W_src, b_src, W_dst, b_dst, _trace=False,
           _return_result=False):
    cfg = _cfg_for(x.shape[0])
    in_maps, plan1, plan2 = _prep_host(x, edge_index, W_src, b_src, W_dst,
                                       b_dst, cfg)
    nc = _build(cfg, plan1, plan2)
    res = _run(nc, in_maps, trace=_trace)
    out = np.concatenate([res.results[c]["out"] for c in range(NCORES)],
                         axis=0)
    if _return_result:
        return out, res
    return out


# revision 10
# speedup vs baseline: 2.4496x; 1.0214x over previous
"""DirGCNConv on 8 Trainium2 NeuronCores via Bass/Tile (v2: scatter-free).

out = (1-a)*(Dout^-1/2 A Din^-1/2 x) @ Wsrc.T + a*(Din^-1/2 A.T Dout^-1/2 x) @ Wdst.T + bias

Per-edge weight separates: w[e] = ao[row[e]] * bi[col[e]], so each direction
is agg[dest] = Sum_{edges} prescaled_x[src], then a per-dest scale + matmul.

v2 strategy (vs v1 gather+scatter-add): edges are sorted by *destination*;
gathered source rows (bf16, dma_gather) are reduced per dest block with
one-hot segment matmuls on the Tensor engine accumulating in PSUM, then
folded into an SBUF-resident [feat x dest] accumulator. This removes all
dma_scatter_add calls — the GpSimd descriptor-generation engine (the
bottleneck) only runs gathers.

SPMD: one program for all 8 cores, so the chunk/matmul schedule is static:
each (window, dest-block) segment is padded to the max token count over
cores. Per-core data (gather indices, one-hot dest columns) differs only in
tensor contents.
"""

import os

import numpy as np
from contextlib import ExitStack

os.environ.setdefault("NEURON_RT_RESET_CORES", "1")

N = 100000
E = 600000
D = 128
NCORES = 8
ALPHA = 0.5

# gather source windows (int16 idx => each <= 32768 rows). Window 0 is
# deliberately small: the first dir-1 gathers wait on its xb prescale.
WBOUNDS = [0, 16384, 44256, 72128, 100000]
CALL = 1024          # max tokens per dma_gather call
SEG = 2048           # prescale segment rows
SEGG = SEG // 128


def _cfg_for(n_nodes):
    assert n_nodes == WBOUNDS[-1]
    nw = len(WBOUNDS) - 1
    nloc = n_nodes // NCORES
    return dict(N=n_nodes, NW=nw, NLOC=nloc,
                NBLK=(nloc + 127) // 128)


def _wrap_idx(arr):
    b = arr.shape[0]
    assert b % 16 == 0
    t = arr.reshape(b // 16, 16).T.copy()
    return np.tile(t, (8, 1)).astype(np.int16)


def _prep_host(x, edge_index, W_src, b_src, W_dst, b_dst, cfg):
    """Pure index reorganization on host -> shared plan + per-core inputs."""
    n, nw, nloc, nblk = cfg["N"], cfg["NW"], cfg["NLOC"], cfg["NBLK"]
    row = np.asarray(edge_index[0], dtype=np.int64)
    col = np.asarray(edge_index[1], dtype=np.int64)

    rp_row = np.zeros(n + 1, dtype=np.int64)
    rp_row[1:] = np.cumsum(np.bincount(row, minlength=n))
    rp_col = np.zeros(n + 1, dtype=np.int64)
    rp_col[1:] = np.cumsum(np.bincount(col, minlength=n))

    def rp_prescale(rp):
        cols = []
        for w in range(nw):
            base = WBOUNDS[w]
            rows_w = WBOUNDS[w + 1] - base
            nseg = rows_w // SEG
            for si in range(nseg):
                cols.append(base + si * SEG
                            + np.arange(128)[:, None] * SEGG
                            + np.arange(SEGG)[None, :])
            r0 = base + nseg * SEG
            rem = rows_w - nseg * SEG
            t1 = rem // 128
            if t1:
                cols.append(r0 + np.arange(t1)[None, :] * 128
                            + np.arange(128)[:, None])
            t2 = rem - t1 * 128
            if t2:
                c = r0 + t1 * 128 + np.arange(128)[:, None]
                cols.append(np.where(c < base + rows_w, c, n))
        idx = np.concatenate(cols, axis=1)
        idx = np.minimum(idx, n)
        lo = rp[idx].astype(np.int32)
        hi = rp[np.minimum(idx + 1, n)].astype(np.int32)
        return lo, hi

    rpa_lo, rpa_hi = rp_prescale(rp_row)
    rpb_lo, rpb_hi = rp_prescale(rp_col)

    def rp_local(rp, c):
        idx = (np.arange(nblk)[None, :] * 128 + np.arange(128)[:, None])
        valid = idx < nloc
        idx = np.minimum(c * nloc + idx, n)
        lo = rp[idx]
        hi = rp[np.minimum(idx + 1, n)]
        hi = np.where(valid, hi, lo)
        return lo.astype(np.int32), hi.astype(np.int32)

    def bucket(dest, src):
        """dest-sorted token streams. Returns (plan, g_list, dloc_list)."""
        core = dest // nloc
        pc = []
        cnt = np.zeros((NCORES, nw, nblk), np.int64)
        for c in range(NCORES):
            m = core == c
            d = (dest[m] - c * nloc).astype(np.int64)
            s = src[m].astype(np.int64)
            w = np.searchsorted(WBOUNDS, s, side="right") - 1
            b = d >> 7
            o = np.lexsort((d, b, w))
            d, s, w, b = (d[o], s[o] - np.asarray(WBOUNDS)[w[o]], w[o],
                          b[o])
            np.add.at(cnt[c], (w, b), 1)
            pc.append((d, s, w, b))
        size_wb = cnt.max(axis=0)                      # [nw, nblk] static
        starts = np.zeros((nw, nblk + 1), np.int64)
        starts[:, 1:] = np.cumsum(size_wb, axis=1)
        wtot = starts[:, -1]
        ntokw = ((wtot + 127) // 128) * 128            # window padded to x128
        win_tok0 = np.zeros(nw, np.int64)
        win_tok0[1:] = np.cumsum(ntokw)[:-1]
        total = int(ntokw.sum())

        # static chunk -> block matmul schedule
        windows = []
        for w in range(nw):
            nch = int(ntokw[w]) // 128
            mms = []                                   # (chunk, block)
            for ci in range(nch):
                lo_t, hi_t = ci * 128, ci * 128 + 128
                for b in range(nblk):
                    if size_wb[w, b] > 0 and starts[w, b] < hi_t \
                            and starts[w, b + 1] > lo_t:
                        mms.append((ci, b))
            calls = [(a, min(CALL, int(ntokw[w]) - a))
                     for a in range(0, int(ntokw[w]), CALL)]
            # psum segment bounds: first/last mm index per block
            seg_first, seg_last = {}, {}
            for j, (ci, b) in enumerate(mms):
                seg_first.setdefault(b, j)
                seg_last[b] = j
            windows.append(dict(tok0=int(win_tok0[w]), ntok=int(ntokw[w]),
                                calls=calls, mms=mms,
                                seg_first=seg_first, seg_last=seg_last))

        g_list, dl_list = [], []
        nmm = sum(len(wd["mms"]) for wd in windows)
        for c in range(NCORES):
            d, s, w, b = pc[c]
            key = w * nblk + b
            gs0 = np.r_[0, np.cumsum(np.bincount(key, minlength=nw * nblk))]
            rank = np.arange(len(d)) - gs0[key]
            pos = win_tok0[w] + starts[w, b] + rank
            g = np.zeros(total, np.int64)
            dl = -np.ones(total, np.int64)
            g[pos] = s
            dl[pos] = d
            # per-mm one-hot dest columns (local id within block or -1)
            cols = np.empty((nmm, 128), np.int16)
            j = 0
            for w2, wd in enumerate(windows):
                dlw = dl[wd["tok0"]:wd["tok0"] + wd["ntok"]].reshape(-1, 128)
                for (ci, b2) in wd["mms"]:
                    r = dlw[ci]
                    cols[j] = np.where((r >= b2 * 128) & (r < (b2 + 1) * 128),
                                       r - b2 * 128, -1).astype(np.int16)
                    j += 1
            g_list.append(_wrap_idx(g.astype(np.int16)))
            dl_list.append(np.ascontiguousarray(cols.T))   # [128, nmm]
        return dict(windows=windows, total=total, nmm=nmm), g_list, dl_list

    plan1, g1, dl1 = bucket(row, col)
    plan2, g2, dl2 = bucket(col, row)

    wsrcT = np.ascontiguousarray(np.asarray(W_src, np.float32).T)
    wdstT = np.ascontiguousarray(np.asarray(W_dst, np.float32).T)
    xf = np.ascontiguousarray(np.asarray(x, np.float32))
    iota = np.tile(np.arange(128, dtype=np.float32)[None, :], (128, 1))

    in_maps = []
    for c in range(NCORES):
        rp1_lo, rp1_hi = rp_local(rp_row, c)
        rp2_lo, rp2_hi = rp_local(rp_col, c)
        in_maps.append({
            "x": xf, "wsrcT": wsrcT, "wdstT": wdstT, "iota": iota,
            "bsrc": np.asarray(b_src, np.float32),
            "bdst": np.asarray(b_dst, np.float32),
            "g1": g1[c], "g2": g2[c],
            "dl1": dl1[c], "dl2": dl2[c],
            "rpa_lo": rpa_lo, "rpa_hi": rpa_hi,
            "rpb_lo": rpb_lo, "rpb_hi": rpb_hi,
            "rp1_lo": rp1_lo, "rp1_hi": rp1_hi,
            "rp2_lo": rp2_lo, "rp2_hi": rp2_hi,
        })
    return in_maps, plan1, plan2


def _build(cfg, plan1, plan2):
    import concourse.tile as tile
    from concourse import bacc, mybir

    dt = mybir.dt
    n, nw = cfg["N"], cfg["NW"]
    nloc, nblk = cfg["NLOC"], cfg["NBLK"]

    nc = bacc.Bacc("TRN2", target_bir_lowering=False, debug=False,
                   num_devices=NCORES)

    x = nc.dram_tensor("x", [n, D], dt.float32, kind="ExternalInput")
    wsrcT = nc.dram_tensor("wsrcT", [D, D], dt.float32, kind="ExternalInput")
    wdstT = nc.dram_tensor("wdstT", [D, D], dt.float32, kind="ExternalInput")
    iota = nc.dram_tensor("iota", [D, D], dt.float32, kind="ExternalInput")
    bsrc = nc.dram_tensor("bsrc", [D], dt.float32, kind="ExternalInput")
    bdst = nc.dram_tensor("bdst", [D], dt.float32, kind="ExternalInput")
    g1 = nc.dram_tensor("g1", [128, plan1["total"] // 16], dt.int16,
                        kind="ExternalInput")
    g2 = nc.dram_tensor("g2", [128, plan2["total"] // 16], dt.int16,
                        kind="ExternalInput")
    dl1 = nc.dram_tensor("dl1", [128, plan1["nmm"]], dt.int16,
                         kind="ExternalInput")
    dl2 = nc.dram_tensor("dl2", [128, plan2["nmm"]], dt.int16,
                         kind="ExternalInput")

    presched = []
    pcols = 0
    for w in range(nw):
        base = WBOUNDS[w]
        rows_w = WBOUNDS[w + 1] - base
        nseg = rows_w // SEG
        steps = []
        for si in range(nseg):
            steps.append(("seg", base + si * SEG, SEG, SEGG))
            pcols += SEGG
        r0 = base + nseg * SEG
        rem = rows_w - nseg * SEG
        t1 = rem // 128
        if t1:
            steps.append(("t1", r0, t1 * 128, t1))
            pcols += t1
        t2 = rem - t1 * 128
        if t2:
            steps.append(("t2", r0 + t1 * 128, t2, 1))
            pcols += 1
        presched.append(steps)

    rpa_lo = nc.dram_tensor("rpa_lo", [128, pcols], dt.int32, kind="ExternalInput")
    rpa_hi = nc.dram_tensor("rpa_hi", [128, pcols], dt.int32, kind="ExternalInput")
    rpb_lo = nc.dram_tensor("rpb_lo", [128, pcols], dt.int32, kind="ExternalInput")
    rpb_hi = nc.dram_tensor("rpb_hi", [128, pcols], dt.int32, kind="ExternalInput")
    rp1_lo = nc.dram_tensor("rp1_lo", [128, nblk], dt.int32, kind="ExternalInput")
    rp1_hi = nc.dram_tensor("rp1_hi", [128, nblk], dt.int32, kind="ExternalInput")
    rp2_lo = nc.dram_tensor("rp2_lo", [128, nblk], dt.int32, kind="ExternalInput")
    rp2_hi = nc.dram_tensor("rp2_hi", [128, nblk], dt.int32, kind="ExternalInput")
    out = nc.dram_tensor("out", [nloc, D], dt.float32, kind="ExternalOutput")

    xbw, xaw = [], []
    for w in range(nw):
        rows_w = WBOUNDS[w + 1] - WBOUNDS[w]
        xbw.append(nc.dram_tensor(f"xb{w}", [rows_w, D], dt.bfloat16))
        xaw.append(nc.dram_tensor(f"xa{w}", [rows_w, D], dt.bfloat16))

    with tile.TileContext(nc) as tc, ExitStack() as ctx:
        const = ctx.enter_context(tc.tile_pool(name="const", bufs=1))

        wsrcT_sb = const.tile([D, D], dt.float32, tag="wsrc")
        nc.sync.dma_start(wsrcT_sb[:], wsrcT.ap())
        wdstT_sb = const.tile([D, D], dt.float32, tag="wdst")
        nc.sync.dma_start(wdstT_sb[:], wdstT.ap())
        iota_sb = const.tile([D, D], dt.float32, tag="iota")
        nc.sync.dma_start(iota_sb[:], iota.ap())

        brow = const.tile([1, 2 * D], dt.float32, tag="brow")
        nc.sync.dma_start(brow[:, 0:D], bsrc.ap().unsqueeze(0))
        nc.sync.dma_start(brow[:, D:2 * D], bdst.ap().unsqueeze(0))
        bsum = const.tile([1, D], dt.float32, tag="bsum")
        nc.vector.tensor_scalar_mul(bsum[:], brow[:, 0:D], 1.0 - ALPHA)
        bs2 = const.tile([1, D], dt.float32, tag="bs2")
        nc.vector.tensor_scalar_mul(bs2[:], brow[:, D:2 * D], ALPHA)
        nc.vector.tensor_add(bsum[:], bsum[:], bs2[:])
        bias_bc = const.tile([D, D], dt.float32, tag="biasbc")
        nc.gpsimd.partition_broadcast(bias_bc[:], bsum[:])

        g1_sb = const.tile([128, plan1["total"] // 16], dt.int16, tag="g1")
        nc.sync.dma_start(g1_sb[:], g1.ap())
        g2_sb = const.tile([128, plan2["total"] // 16], dt.int16, tag="g2")
        nc.sync.dma_start(g2_sb[:], g2.ap())

        dlf1 = const.tile([128, plan1["nmm"]], dt.float32, tag="dlf1")
        dlf2 = const.tile([128, plan2["nmm"]], dt.float32, tag="dlf2")

        def invsqrt_chain(pool, lo_ap, hi_ap, cols, tag, scale=None,
                          res_pool=None):
            res_pool = res_pool or pool
            lo_t = pool.tile([128, cols], dt.int32, tag=tag + "lo")
            nc.sync.dma_start(lo_t[:], lo_ap)
            hi_t = pool.tile([128, cols], dt.int32, tag=tag + "hi")
            nc.sync.dma_start(hi_t[:], hi_ap)
            deg_i = pool.tile([128, cols], dt.int32, tag=tag + "di")
            nc.vector.tensor_sub(deg_i[:], hi_t[:], lo_t[:])
            deg_f = pool.tile([128, cols], dt.float32, tag=tag + "df")
            nc.vector.tensor_copy(deg_f[:], deg_i[:])
            mask = pool.tile([128, cols], dt.float32, tag=tag + "mk")
            mul = scale if scale is not None else 1.0
            nc.vector.tensor_scalar(mask[:], deg_f[:], 1.0, mul,
                                    mybir.AluOpType.min, mybir.AluOpType.mult)
            dmax = pool.tile([128, cols], dt.float32, tag=tag + "dm")
            nc.vector.tensor_scalar_max(dmax[:], deg_f[:], 1.0)
            rec = pool.tile([128, cols], dt.float32, tag=tag + "rc")
            nc.vector.reciprocal(rec[:], dmax[:])
            sq = pool.tile([128, cols], dt.float32, tag=tag + "sq")
            nc.scalar.sqrt(sq[:], rec[:])
            res = res_pool.tile([128, cols], dt.float32, tag=tag + "rs")
            nc.vector.tensor_mul(res[:], sq[:], mask[:])
            return res

        with tc.tile_pool(name="chainscratch", bufs=1) as csp:
            a_vec = invsqrt_chain(csp, rp1_lo.ap(), rp1_hi.ap(), nblk, "av",
                                  scale=1.0 - ALPHA, res_pool=const)
            b_vec = invsqrt_chain(csp, rp2_lo.ap(), rp2_hi.ap(), nblk, "bv",
                                  scale=ALPHA, res_pool=const)
            b_full = invsqrt_chain(csp, rpb_lo.ap(), rpb_hi.ap(), pcols,
                                   "bf", res_pool=const)
            a_full = invsqrt_chain(csp, rpa_lo.ap(), rpa_hi.ap(), pcols,
                                   "af", res_pool=const)
            di1 = csp.tile([128, plan1["nmm"]], dt.int16, tag="di1")
            nc.sync.dma_start(di1[:], dl1.ap())
            nc.vector.tensor_copy(dlf1[:], di1[:])
            di2 = csp.tile([128, plan2["nmm"]], dt.int16, tag="di2")
            nc.sync.dma_start(di2[:], dl2.ap())
            nc.vector.tensor_copy(dlf2[:], di2[:])

        # SBUF accumulators [feat x dest], one per direction
        agg1_sb = const.tile([128, nblk * 128], dt.float32, tag="agg1")
        agg2_sb = const.tile([128, nblk * 128], dt.float32, tag="agg2")
        for agg in (agg1_sb, agg2_sb):
            off = 0
            while off < nblk * 128:
                csz = min(4096, nblk * 128 - off)
                nc.vector.memset(agg[:, off:off + csz], 0.0)
                off += csz

        gpool = ctx.enter_context(tc.tile_pool(name="gat", bufs=4))
        spool = ctx.enter_context(tc.tile_pool(name="sb", bufs=4))
        epsum = ctx.enter_context(tc.tile_pool(name="eps", bufs=4,
                                               space="PSUM"))

        def prescale_window(pp, w, which, ccur):
            dest, sv = (xbw[w], b_full) if which == "b" else (xaw[w], a_full)
            for kind_, r0, nrows, ncols in presched[w]:
                base = WBOUNDS[w]
                cs = slice(ccur, ccur + ncols)
                ccur += ncols
                if kind_ == "t2":
                    xt = pp.tile([nrows, D], dt.float32, tag="pxt2")
                    nc.sync.dma_start(xt[:], x.ap()[r0:r0 + nrows, :])
                    ot = pp.tile([nrows, D], dt.bfloat16, tag="pot2")
                    nc.scalar.mul(ot[:], xt[:], sv[0:nrows, cs])
                    nc.sync.dma_start(
                        dest.ap()[r0 - base:r0 - base + nrows, :], ot[:])
                    continue
                wrap = "(p g) d -> p g d" if kind_ == "seg" \
                    else "(g p) d -> p g d"
                xs = x.ap()[r0:r0 + nrows, :].rearrange(wrap, p=128)
                xt = pp.tile([128, ncols, D], dt.float32, tag="pxt")
                nc.sync.dma_start(xt[:], xs)
                ex = pp.tile([128, ncols, D], dt.bfloat16, tag="pex")
                nc.vector.tensor_tensor(
                    ex[:], sv[:, cs].unsqueeze(2).to_broadcast(
                        [128, ncols, D]),
                    xt[:], mybir.AluOpType.mult)
                dv = dest.ap()[r0 - base:r0 - base + nrows, :] \
                    .rearrange(wrap, p=128)
                nc.sync.dma_start(dv, ex[:])
            return ccur

        def edge_window(plan, w, g_sb, dlf, srcw, agg_sb, mm0, dtag,
                        final_cb=None):
            wd = plan["windows"][w]
            xs = srcw[w].ap()
            tiles = {}
            for (a, ln) in wd["calls"]:
                xt = gpool.tile([128, CALL // 128, D], dt.bfloat16,
                                tag="xt" + dtag)
                o = wd["tok0"] + a
                gi = g_sb[:, o // 16:(o + ln) // 16]
                nc.gpsimd.dma_gather(xt[:, 0:ln // 128, :], xs, gi, ln, ln, D)
                for g in range(ln // 128):
                    tiles[a // 128 + g] = (xt, g)
            mms = wd["mms"]
            # batched one-hot builds (8 consecutive mm columns per op)
            sb_tiles = []
            for j0 in range(0, len(mms), 8):
                jn = min(8, len(mms) - j0)
                st = spool.tile([128, 8, D], dt.bfloat16, tag="st")
                nc.vector.tensor_tensor(
                    st[:, 0:jn, :],
                    dlf[:, mm0 + j0:mm0 + j0 + jn].unsqueeze(2)
                    .to_broadcast([128, jn, D]),
                    iota_sb[:].unsqueeze(1).to_broadcast([128, jn, D]),
                    mybir.AluOpType.is_equal)
                sb_tiles.append(st)
            active = {}
            for j, (ci, b) in enumerate(mms):
                xt, g = tiles[ci]
                st = sb_tiles[j // 8]
                if b not in active:
                    active[b] = epsum.tile([128, D], dt.float32, tag="ep",
                                           name="ep")
                ps = active[b]
                nc.tensor.matmul(ps[:], lhsT=xt[:, g, :], rhs=st[:, j % 8, :],
                                 start=(j == wd["seg_first"][b]),
                                 stop=(j == wd["seg_last"][b]))
                if j == wd["seg_last"][b]:
                    nc.vector.tensor_add(
                        agg_sb[:, b * 128:(b + 1) * 128],
                        agg_sb[:, b * 128:(b + 1) * 128], ps[:])
                    del active[b]
                    if final_cb is not None:
                        final_cb(b)
            return mm0 + len(mms)

        # final per-block output, emitted as soon as a block's accumulators
        # are complete (interleaved into the last window's dir-2 stream)
        fp = ctx.enter_context(tc.tile_pool(name="fin", bufs=3))
        fps = ctx.enter_context(tc.tile_pool(name="fps", bufs=2,
                                             space="PSUM"))

        def final_block(k):
            ks = slice(k * 128, (k + 1) * 128)
            p1 = fps.tile([128, D], dt.float32, tag="p1", name="p1")
            nc.tensor.matmul(p1[:], lhsT=agg1_sb[:, ks], rhs=wsrcT_sb[:],
                             start=True, stop=True)
            p2 = fps.tile([128, D], dt.float32, tag="p2", name="p2")
            nc.tensor.matmul(p2[:], lhsT=agg2_sb[:, ks], rhs=wdstT_sb[:],
                             start=True, stop=True)
            o1 = fp.tile([128, D], dt.float32, tag="o1", name="o1")
            nc.scalar.mul(o1[:], p1[:], a_vec[:, k:k + 1])
            o2 = fp.tile([128, D], dt.float32, tag="o2", name="o2")
            nc.scalar.mul(o2[:], p2[:], b_vec[:, k:k + 1])
            fin = fp.tile([128, D], dt.float32, tag="fin", name="fin")
            nc.vector.tensor_add(fin[:], o1[:], o2[:])
            nc.vector.tensor_add(fin[:], fin[:], bias_bc[:])
            rows = min(128, nloc - k * 128)
            nc.sync.dma_start(out.ap()[k * 128:k * 128 + rows, :],
                              fin[0:rows, :])

        emitted = set()

        def final_cb(b):
            if b not in emitted:
                emitted.add(b)
                final_block(b)

        # interleave: xb prescale feeds dir-1 gathers, xa feeds dir-2;
        # window w+1 prescale is emitted before window w's edge stream
        with tc.tile_pool(name="prescale", bufs=2) as pp:
            ccb, cca, mm1, mm2 = [0], [0], [0], [0]

            def pre_b(w):
                ccb[0] = prescale_window(pp, w, "b", ccb[0])

            def pre_a(w):
                cca[0] = prescale_window(pp, w, "a", cca[0])

            def e1(w, cb=None):
                mm1[0] = edge_window(plan1, w, g1_sb, dlf1, xbw, agg1_sb,
                                     mm1[0], "1", cb)

            def e2(w, cb=None):
                mm2[0] = edge_window(plan2, w, g2_sb, dlf2, xaw, agg2_sb,
                                     mm2[0], "2", cb)

            pre_b(0)
            if nw > 1:
                pre_b(1)
            pre_a(0)
            e1(0)
            if nw > 1:
                pre_a(1)
            e2(0)
            for w in range(1, nw - 1):
                pre_b(w + 1)
                e1(w)
                pre_a(w + 1)
                e2(w)
            if nw > 1:
                e1(nw - 1)
                e2(nw - 1, final_cb)
        for k in range(nblk):
            if k not in emitted:
                emitted.add(k)
                final_block(k)

    nc.compile()
    return nc


def _install_ntff_shim():
    """This image's antenv lacks axon_hooks; inject it so trace=True works."""
    import sys
    import types
    try:
        from antenv import axon_hooks  # noqa: F401
        return
    except ImportError:
        pass
    try:
        import antenv
        from trn_agent_boot.trn_boot import _ntff_profile_via_ctypes
        mod = types.ModuleType("antenv.axon_hooks")
        holder = [None]
        mod.set_axon_ntff_profile_hook = lambda h: holder.__setitem__(0, h)
        mod.get_axon_ntff_profile_hook = lambda: holder[0]
        sys.modules["antenv.axon_hooks"] = mod
        antenv.axon_hooks = mod
        mod.set_axon_ntff_profile_hook(
            _ntff_profile_via_ctypes("/opt/axon/libaxon_pjrt.so"))
    except Exception as e:  # profiling is best-effort
        print("ntff shim failed:", e)


def _run(nc, in_maps, trace=False):
    from concourse.bass_utils import run_bass_kernel_spmd
    kw = {}
    if trace:
        _install_ntff_shim()
        kw = dict(trace=True, trace_cores=list(range(NCORES)))
    return run_bass_kernel_spmd(nc, in_maps, list(range(NCORES)), **kw)


def kernel(x, edge_index, W_src, b_src, W_dst, b_dst, _trace=False,
           _return_result=False):
    cfg = _cfg_for(x.shape[0])
    in_maps, plan1, plan2 = _prep_host(x, edge_index, W_src, b_src, W_dst,
                                       b_dst, cfg)
    nc = _build(cfg, plan1, plan2)
    res = _run(nc, in_maps, trace=_trace)
    out = np.concatenate([res.results[c]["out"] for c in range(NCORES)],
                         axis=0)
    if _return_result:
        return out, res
    return out
